# revision 30
# baseline (speedup 1.0000x reference)
"""MoE-routed low-rank attention (nn_NeuronCircuit_28930899706061) on 8 TRN2 cores.

Sharding: core c in 0..7 -> batch b = c//4, token-shard q4 = c%4 (512 tokens)
for the compress/routing phase; head group heads [4*q4, 4*q4+4) of batch b for
the attention phase. h^T tensors for all 3 routers are all-gathered in ONE
collective within each 4-core batch group; each core emits a partial [S, D]
output (its 4 heads' contribution through wO) and the host sums the 4 partials
per batch.

Attention computes scores TRANSPOSED ([k, q] with k on partitions) so the
softmax-weighted AV needs no P-transpose: AV contracts k on partitions
directly, and the softmax denominator comes for free from a ones-column
appended to V (row 64/63 of the AV PSUM accumulator). Normalization is a
PE outer-product broadcast of 1/z plus one DVE multiply per head.

All weight transposes are done host-side so every device DMA is contiguous.
"""

import os

import numpy as np

# ablation knobs (default = fast path); set to "0" to fall back
KNOB_TA = os.environ.get("KNOB_TA", "1") == "1"  # PE transpose-accum combine
KNOB_SHIFT = os.environ.get("KNOB_SHIFT", "1") == "1"  # K=128 wO w/ PE shift
KNOB_AFF = os.environ.get("KNOB_AFF", "1") == "1"  # gpsimd affine_select mask

B, S, D, H, RANK, NCMP = 2, 2048, 1024, 16, 128, 16
DH = D // H  # 64
N_CORES = 8
SHARD = S // 4  # 512 tokens per core in phase 1
HPC = 4  # heads per core
QT_TILES = S // 128  # 16 q tiles
KT_TILES = S // 128  # 16 k tiles

_RUNNERS: dict = {}


def _split_multi_waits(nc, mybir):
    """This toolchain's walrus rejects any instruction carrying >1 sync wait
    ("Too many sync wait commands"); hoist excess waits onto same-engine nops
    inserted immediately before the instruction."""
    cnt = 0
    for f in nc.m.functions:
        for blk in f.blocks:
            il = blk.instructions
            out = []
            changed = False
            for inst in il:
                si = inst.sync_info
                waits = list(si.on_wait or []) if si else []
                if len(waits) > 1:
                    for w in waits[:-1]:
                        cnt += 1
                        nop = mybir.InstNoOp(
                            name=f"wsplit-{cnt}",
                            engine=inst.engine,
                            sync_info=mybir.SyncInfo(on_wait=[w], on_update=[]),
                        )
                        nc.register_instruction(nop)
                        out.append(nop)
                    inst.sync_info = mybir.SyncInfo(
                        on_wait=[waits[-1]], on_update=list(si.on_update or [])
                    )
                    changed = True
                out.append(inst)
            if changed:
                il[:] = out


def _make_tc_class(tile, mybir):
    class TC(tile.TileContext):
        def __exit__(self, *exc):
            ret = super().__exit__(*exc)
            if exc[0] is None:
                _split_multi_waits(self.nc, mybir)
            return ret

    return TC


_TRI = np.tril(np.ones((128, 128), dtype=bool))  # keep k<=q in [k,q] layout is triu


def _mask_plan(maskb):
    """Per q-tile: (nkt, diag_kt, customs) in TRANSPOSED [k, q] tile terms.

    nkt: number of live 128-wide k tiles; diag_kt: k-tile index that is
    exactly the causal triangle (k<=q kept), or None; customs: list of
    (kt, mask_tile_idx) for other partial tiles.  Returns (plan, tiles)
    with tiles a [nt, 128, 128] f32 array of ADDITIVE masks in [k, q]
    orientation.
    """
    tiles = []
    tile_ids = {}
    plan = []
    for qi in range(QT_TILES):
        rows = maskb[qi * 128 : (qi + 1) * 128]  # [128 q, S k]
        nkt = 0
        for kt in range(KT_TILES):
            if rows[:, kt * 128 : (kt + 1) * 128].any():
                nkt = kt + 1
        diag_kt = None
        customs = []
        for kt in range(nkt):
            sub = rows[:, kt * 128 : (kt + 1) * 128]  # [q, k]
            if sub.all():
                continue
            subT = sub.T  # [k, q]
            if KNOB_AFF and diag_kt is None and (subT == _TRI.T).all():
                # exactly "keep k <= q within tile"
                diag_kt = kt
                continue
            add = np.where(subT, np.float32(0), np.float32(-1e30))
            key = add.tobytes()
            if key not in tile_ids:
                tile_ids[key] = len(tiles)
                tiles.append(add)
            customs.append((kt, tile_ids[key]))
        plan.append((nkt, diag_kt, tuple(customs)))
    nt = len(tiles)
    tiles_arr = (
        np.stack(tiles).astype(np.float32)
        if nt
        else np.zeros((0, 128, 128), np.float32)
    )
    return tuple(plan), tiles_arr


def _bcast_mid(bass, ap, n):
    """[P, K] AP -> [P, n, K] AP with a step-0 middle dim (free broadcast)."""
    dims = [list(x) for x in ap.ap]
    return bass.AP(
        tensor=ap.tensor, offset=ap.offset, ap=[dims[0], [0, n]] + dims[1:]
    )


def _build(plan, nt, repeat=1):
    import concourse.bass as bass
    import concourse.mybir as mybir
    import concourse.tile as tile
    from concourse.bass import ts
    from concourse.masks import make_identity

    f32 = mybir.dt.float32
    bf16 = mybir.dt.bfloat16
    Exp = mybir.ActivationFunctionType.Exp
    TC = _make_tc_class(tile, mybir)

    nc = bass.Bass(num_devices=N_CORES)
    xT_d = nc.dram_tensor("xT", [8, 128, SHARD], bf16, kind="ExternalInput")
    cflat_d = nc.dram_tensor("cflat", [8, 128, RANK * NCMP], bf16, kind="ExternalInput")
    routersT_d = nc.dram_tensor("routersT", [8, 128, 48], bf16, kind="ExternalInput")
    wqkvT_d = nc.dram_tensor("wqkvT", [3, 128, HPC * DH], bf16, kind="ExternalInput")
    wOT_d = nc.dram_tensor("wOT", [2, 128, D], bf16, kind="ExternalInput")
    dmask_d = (
        nc.dram_tensor("dmask", [nt, 128, 128], f32, kind="ExternalInput")
        if nt
        else None
    )
    out_d = nc.dram_tensor("out", [S, D], bf16, kind="ExternalOutput")

    groups = [[0, 1, 2, 3], [4, 5, 6, 7]]
    NT4 = SHARD // 128  # 4 s-tiles per core in phase 1

    with TC(nc) as tc:
      for _rep in range(repeat):
        with (
            tc.tile_pool(name="sb", bufs=1) as sbp,
            tc.tile_pool(name="wk2", bufs=2) as wk2,
            tc.tile_pool(name="wk4", bufs=4) as wk4,
            tc.tile_pool(name="dram", bufs=1, space="DRAM") as dramp,
        ):
            ident16 = sbp.tile([128, 128], bf16)
            make_identity(nc, ident16)
            if not KNOB_TA:
                identf = sbp.tile([128, 128], f32)
                make_identity(nc, identf)
            ones64 = sbp.tile([128, 64], bf16)
            nc.vector.memset(ones64[:], 1.0)
            cbias = sbp.tile([128, 1], f32)
            nc.vector.memset(cbias[:], -20.0)
            cc_in = dramp.tile([128, 3 * SHARD], bf16, name="cc_in")
            cc_out = dramp.tile([4, 128, 3 * SHARD], bf16, name="cc_out")

            # phase-1 inputs first on the DMA queue (they gate the pipeline)
            xt = sbp.tile([128, 8, SHARD], bf16)
            rt = sbp.tile([128, 8, 48], bf16)
            cfl = sbp.tile([128, 8, RANK * NCMP], bf16)
            nc.sync.dma_start(xt[:], xT_d[:].rearrange("d p s -> p d s"))
            nc.sync.dma_start(rt[:], routersT_d[:].rearrange("d p s -> p d s"))
            for dk in range(8):
                nc.sync.dma_start(
                    cfl[:, dk, ts(0, 512)], cflat_d[dk][:, ts(0, 512)]
                )
            # attention-side constant loads
            wq = sbp.tile([128, 3, HPC * DH], bf16)
            for r in range(3):
                nc.sync.dma_start(wq[:, r, :], wqkvT_d[r])
            if KNOB_SHIFT:
                wo = sbp.tile([128, 2, D], bf16)
                for k in range(2):
                    nc.sync.dma_start(wo[:, k, :], wOT_d[k])
            else:
                # per-head wO rows on partitions 0..63 (K=64 accumulation)
                wo4 = sbp.tile([64, HPC, D], bf16)
                for k in range(2):
                    nc.sync.dma_start(
                        wo4[:, 2 * k : 2 * k + 2, :],
                        wOT_d[k].rearrange("(h p) e -> p h e", h=2),
                    )
            # remaining compress-weight chunks
            for ch in range(1, 4):
                for dk in range(8):
                    nc.sync.dma_start(
                        cfl[:, dk, ts(ch, 512)], cflat_d[dk][:, ts(ch, 512)]
                    )
            if nt:
                masks = sbp.tile([128, nt, 128], f32)
                for t in range(nt):
                    nc.sync.dma_start(masks[:, t, :], dmask_d[t])
            # V' tile: per (k-tile, head): [V | 1]; the ones column makes
            # row 64 of the AV accumulator the softmax denominator z
            vV = sbp.tile([128, KT_TILES, HPC, 65], bf16)
            nc.gpsimd.memset(vV[:, :, :, 64:65], 1.0)

            with (
                tc.tile_pool(name="p1y", bufs=2, space="PSUM") as p1y,
                tc.tile_pool(name="p1w", bufs=1, space="PSUM") as p1w,
                tc.tile_pool(name="p1t", bufs=2, space="PSUM") as p1t,
            ):
                # ------------- phase 1: routing + compress -------------

                ys = [
                    sbp.tile([128, RANK * NCMP], bf16, name=f"y{t}")
                    for t in range(NT4)
                ]
                w3nb = sbp.tile([128, NT4, 48], bf16)

                def emit_combine(t):
                    """Weighted neuron-sum + transpose + gather-staging for one
                    finished s-tile.  Deferred one tile so the PE never stalls
                    on the DVE multiply."""
                    for r in range(3):
                        yv = ys[t][:].rearrange("p (r n) -> p r n", n=NCMP)
                        tmp = wk2.tile([128, RANK * NCMP], bf16, tag="tmp")
                        tv = tmp[:].rearrange("p (r n) -> p r n", n=NCMP)
                        wb = _bcast_mid(bass, w3nb[:, t, ts(r, 16)], RANK)
                        nc.vector.tensor_mul(tv, yv, wb)
                        if not KNOB_TA:
                            # baseline-style: DVE grouped reduce + f32 transpose
                            hT = p1t.tile([128, 128], f32, tag="hT")
                            h = wk4.tile([128, RANK], f32, tag="h")
                            nc.vector.tensor_reduce(
                                h[:], tv, axis=mybir.AxisListType.X,
                                op=mybir.AluOpType.add,
                            )
                            nc.tensor.matmul(
                                hT[:], h[:], identf[:], is_transpose=True,
                                start=True, stop=True, skip_group_check=True,
                            )
                        elif r < 2:
                            # DVE grouped reduce + single bf16 transpose
                            hT = p1t.tile([128, 128], bf16, tag="hT")
                            h = wk4.tile([128, RANK], f32, tag="h")
                            nc.vector.tensor_reduce(
                                h[:], tv, axis=mybir.AxisListType.X,
                                op=mybir.AluOpType.add,
                            )
                            hb = wk4.tile([128, RANK], bf16, tag="hb")
                            nc.scalar.copy(hb[:], h[:])
                            nc.tensor.matmul(
                                hT[:], hb[:], ident16[:], is_transpose=True,
                                start=True, stop=True, skip_group_check=True,
                            )
                        else:
                            hT = p1t.tile([128, 128], bf16, tag="hT")
                            # PE-fused: 16 accumulating transpose passes sum
                            # over neurons and transpose in one go
                            for n in range(NCMP):
                                nc.tensor.matmul(
                                    hT[:], tv[:, :, n], ident16[:],
                                    is_transpose=True,
                                    start=(n == 0), stop=(n == NCMP - 1),
                                    skip_group_check=True,
                                )
                        hTs = wk4.tile([128, 128], bf16, tag="hTs")
                        nc.scalar.copy(hTs[:], hT[:])
                        nc.sync.dma_start(
                            cc_in[:, r * SHARD + t * 128 : r * SHARD + (t + 1) * 128],
                            hTs[:],
                        )

                for t in range(NT4):
                    w3ps = p1w.tile([128, 48], f32, tag="w3ps")
                    for dk in range(8):
                        nc.tensor.matmul(
                            w3ps[:], xt[:, dk, ts(t, 128)], rt[:, dk, :],
                            start=(dk == 0), stop=(dk == 7),
                        )
                    for r in range(3):
                        # no max-subtraction: router logits are O(1)
                        e3 = wk4.tile([128, 16], f32, tag="e3")
                        z3 = wk4.tile([128, 1], f32, tag="z3")
                        nc.scalar.activation(
                            e3[:], w3ps[:, ts(r, 16)], Exp,
                            bias=0.0, scale=1.0, accum_out=z3[:],
                        )
                        rz3 = wk4.tile([128, 1], f32, tag="rz3")
                        nc.vector.reciprocal(rz3[:], z3[:])
                        nc.vector.tensor_scalar_mul(
                            w3nb[:, t, ts(r, 16)], e3[:], rz3[:]
                        )

                    # y[s, (r, n)] = x @ C  (r outer, n inner), 2 chunks/bank-pair
                    for cp in range(2):
                        yp = p1y.tile([128, 2, 512], f32, tag="yp")
                        for c2 in range(2):
                            ch = cp * 2 + c2
                            for dk in range(8):
                                nc.tensor.matmul(
                                    yp[:, c2, :], xt[:, dk, ts(t, 128)],
                                    cfl[:, dk, ts(ch, 512)],
                                    start=(dk == 0), stop=(dk == 7),
                                )
                        nc.scalar.copy(ys[t][:, ts(cp, 1024)], yp[:])
                    if t > 0:
                        emit_combine(t - 1)
                emit_combine(NT4 - 1)
                # single merged AllGather for all 3 routers
                nc.gpsimd.collective_compute(
                    "AllGather", mybir.AluOpType.bypass,
                    replica_groups=groups,
                    ins=[cc_in[:]], outs=[cc_out[:]],
                )

            # ------------- expand: Q^T, K^T, V' -------------
            hxt = sbp.tile([128, 3, S], bf16)
            for r in range(3):
                srcap = bass.AP(
                    tensor=cc_out.tensor,
                    offset=cc_out.offset + r * SHARD,
                    ap=[[3 * SHARD, 128], [128 * 3 * SHARD, 4], [1, SHARD]],
                )
                nc.sync.dma_start(hxt[:, r, :], srcap)

            QT = sbp.tile([128, 2, S], bf16)
            KT = sbp.tile([128, 2, S], bf16)
            with tc.tile_pool(name="qkps", bufs=3, space="PSUM") as qkps:
                for r, dst in ((0, QT), (1, KT)):
                    for pair in range(2):
                        for ch in range(4):
                            ps = qkps.tile([128, 512], f32, tag="qk")
                            nc.tensor.matmul(
                                ps[:], wq[:, r, ts(pair, 128)],
                                hxt[:, r, ts(ch, 512)],
                                start=True, stop=True,
                            )
                            nc.vector.tensor_copy(dst[:, pair, ts(ch, 512)], ps[:])
                for st in range(KT_TILES):
                    vps = qkps.tile([128, HPC * DH], f32, tag="qk")
                    nc.tensor.matmul(
                        vps[:], hxt[:, 2, ts(st, 128)], wq[:, 2, :],
                        start=True, stop=True,
                    )
                    nc.vector.tensor_copy(
                        vV[:, st, :, 0:64],
                        vps[:].rearrange("p (h d) -> p h d", d=DH),
                    )

            # ------------- attention + wO -------------
            with (
                tc.tile_pool(name="stp", bufs=2, space="PSUM") as stp,
                tc.tile_pool(name="avp", bufs=2, space="PSUM") as avp,
                tc.tile_pool(name="bcp", bufs=1, space="PSUM") as bcp,
                tc.tile_pool(name="opp", bufs=1, space="PSUM") as opp,
            ):

                def emit_norm(qi, av):
                    """Normalize a finished q-tile into a pair-stacked aoT.
                    Deferred into the next q-tile's issue window so the queues
                    never stall on the AV->recip->bc->mul chain."""
                    rzt = wk2.tile([128, HPC, 128], bf16, tag="rzt")
                    # 1/z for all 4 heads in one op (z rows on partition 64)
                    with nc.allow_low_precision(
                        reason="1/z in bf16: 0.4% relative, well under tol"
                    ):
                        nc.vector.reciprocal(rzt[64:65, :, :], av[64:65, :, :])
                    # broadcast 1/z to 64 rows via PE outer product, then
                    # normalize each head's [64, q] tile into bf16.  The same
                    # PSUM bank hosts the odd-head partition-shift staging on
                    # partitions 64..127 (disjoint bytes).
                    bsh = bcp.tile([128, HPC, 128], f32, tag="bc")
                    for h in range(HPC):
                        nc.tensor.matmul(
                            bsh[0:64, h, :], ones64[64:65, :], rzt[64:65, h, :],
                            start=True, stop=True, skip_group_check=True,
                        )
                    bcs = wk2.tile([64, HPC, 128], bf16, tag="bcs")
                    nc.vector.tensor_copy(bcs[:], bsh[0:64, :, :])
                    # pair-stacked aoT for a K=128 wO: even heads' normalized
                    # tiles are written in place (partitions 0..63); odd heads
                    # go to a scratch tile and a PE identity-matmul moves both
                    # to partitions 64..127 (compute engines are lane-locked,
                    # but the PE can write any PSUM partition range)
                    if KNOB_SHIFT:
                        aoT = wk2.tile([128, 2, 128], bf16, tag="aoT")
                        aodd = wk2.tile([64, 2, 128], bf16, tag="aodd")
                        for pair in range(2):
                            nc.vector.tensor_mul(
                                aodd[:, pair, :], av[0:64, 2 * pair + 1, :],
                                bcs[:, 2 * pair + 1, :],
                            )
                        for pair in range(2):
                            nc.tensor.matmul(
                                bsh[64:128, pair, :], ident16[0:64, 0:64],
                                aodd[:, pair, :],
                                start=True, stop=True, skip_group_check=True,
                            )
                        nc.vector.tensor_copy(
                            aoT[64:128, :, :], bsh[64:128, 0:2, :]
                        )
                        for pair in range(2):
                            nc.vector.tensor_mul(
                                aoT[0:64, pair, :], av[0:64, 2 * pair, :],
                                bcs[:, 2 * pair, :],
                            )
                    else:
                        aoT = wk2.tile([64, HPC, 128], bf16, tag="aoT")
                        for h in range(HPC):
                            nc.vector.tensor_mul(
                                aoT[:, h, :], av[0:64, h, :], bcs[:, h, :]
                            )
                    return aoT

                def emit_wo(qi, aoT):
                    for e in range(2):
                        ops = opp.tile([128, 512], f32, tag="o")
                        if KNOB_SHIFT:
                            for pair in range(2):
                                nc.tensor.matmul(
                                    ops[:], aoT[:, pair, :],
                                    wo[:, pair, ts(e, 512)],
                                    start=(pair == 0), stop=(pair == 1),
                                    skip_group_check=True,
                                )
                        else:
                            for h in range(HPC):
                                nc.tensor.matmul(
                                    ops[:], aoT[:, h, :], wo4[:, h, ts(e, 512)],
                                    start=(h == 0), stop=(h == 3),
                                    skip_group_check=True,
                                )
                        osb = wk2.tile([128, 512], bf16, tag="osb")
                        nc.vector.tensor_copy(osb[:], ops[:])
                        nc.sync.dma_start(out_d[ts(qi, 128), ts(e, 512)], osb[:])

                def emit_scores(qi, h, nkt, diag_kt, customs):
                    """QK^T + exp for one (q-tile, head); returns the P tile."""
                    pair, doff = h // 2, (h % 2) * 64
                    p_sb = wk2.tile([128, KT_TILES, 128], bf16, tag="p")
                    for kb in range(0, nkt, 8):
                        kw = min(8, nkt - kb)
                        st_ps = stp.tile([128, 8, 128], f32, tag="st")
                        for j in range(kw):
                            nc.tensor.matmul(
                                st_ps[:, j, :],
                                KT[doff : doff + 64, pair, ts(kb + j, 128)],
                                QT[doff : doff + 64, pair, ts(qi, 128)],
                                start=True, stop=True,
                                skip_group_check=True,
                            )
                        for kt, mi in customs:
                            if kb <= kt < kb + kw:
                                nc.vector.tensor_add(
                                    st_ps[:, kt - kb, :],
                                    st_ps[:, kt - kb, :],
                                    masks[:, mi, :],
                                )
                        # exp(score - 20); the shift cancels in the
                        # softmax normalization
                        nc.scalar.activation(
                            p_sb[:, kb : kb + kw, :], st_ps[:, 0:kw, :],
                            Exp, bias=cbias[:], scale=1.0,
                        )
                        if diag_kt is not None and kb <= diag_kt < kb + kw:
                            # zero P where k > q (keep -p + c >= 0)
                            nc.gpsimd.affine_select(
                                p_sb[:, diag_kt, :], p_sb[:, diag_kt, :],
                                pattern=[[1, 128]],
                                compare_op=mybir.AluOpType.is_ge,
                                fill=0.0,
                                base=0,
                                channel_multiplier=-1,
                            )
                    return p_sb

                def emit_av(av, h, nkt, p_sb):
                    for kt in range(nkt):
                        nc.tensor.matmul(
                            av[:, h, :],
                            vV[:, kt, h, :],
                            p_sb[:, kt, :],
                            start=(kt == 0), stop=(kt == nkt - 1),
                            skip_group_check=True,
                        )

                # two-level software pipeline: AV lags scores by one head; the
                # normalize chain for q-tile i is issued at the start of tile
                # i+1 and its wO two heads later, so PE never waits on the Act
                # exp or the DVE normalize chain.
                pend_norm = None
                pend_wo = None
                pend_av = None
                for qi in range(QT_TILES):
                    nkt, diag_kt, customs = plan[qi]
                    av = avp.tile([65, HPC, 128], f32, tag="av")
                    for h in range(HPC):
                        p_sb = emit_scores(qi, h, nkt, diag_kt, customs)
                        if pend_av is not None:
                            emit_av(*pend_av)
                        pend_av = (av, h, nkt, p_sb)
                        if h == 0 and pend_norm is not None:
                            pqi, pav = pend_norm
                            pend_wo = (pqi, emit_norm(pqi, pav))
                            pend_norm = None
                        elif h == 2 and pend_wo is not None:
                            emit_wo(*pend_wo)
                            pend_wo = None
                    pend_norm = (qi, av)
                emit_av(*pend_av)
                if pend_wo is not None:
                    emit_wo(*pend_wo)
                pqi, pav = pend_norm
                emit_wo(pqi, emit_norm(pqi, pav))

    return nc


def _make_runner(plan, nt, repeat=1):
    """Compile the graph once and return fn(in_maps) -> list of out arrays."""
    import jax
    import numpy as np
    from jax.sharding import Mesh, PartitionSpec
    from jax.experimental.shard_map import shard_map
    import concourse.bass2jax as bass2jax
    import concourse.mybir as mybir

    nc = _build(plan, nt, repeat=repeat)
    bass2jax.install_neuronx_cc_hook()

    partition_name = nc.partition_id_tensor.name if nc.partition_id_tensor else None
    in_names, out_names, out_avals = [], [], []
    for alloc in nc.m.functions[0].allocations:
        if not isinstance(alloc, mybir.MemoryLocationSet):
            continue
        name = alloc.memorylocations[0].name
        if alloc.kind == "ExternalInput":
            if name != partition_name:
                in_names.append(name)
        elif alloc.kind == "ExternalOutput":
            out_names.append(name)
            out_avals.append(
                jax.core.ShapedArray(
                    tuple(alloc.tensor_shape), mybir.dt.np(alloc.dtype)
                )
            )
    all_names = in_names + out_names
    if partition_name is not None:
        all_names = all_names + [partition_name]

    def _body(*args):
        operands = list(args)
        if partition_name is not None:
            operands.append(bass2jax.partition_id_tensor())
        outs = bass2jax._bass_exec_p.bind(
            *operands,
            out_avals=tuple(out_avals),
            in_names=tuple(all_names),
            out_names=tuple(out_names),
            lowering_input_output_aliases=(),
            sim_require_finite=True,
            sim_require_nnan=True,
            nc=nc,
        )
        return tuple(outs)

    devices = jax.devices()[:N_CORES]
    mesh = Mesh(np.asarray(devices), ("core",))
    SHARED = {"cflat", "routersT", "dmask"}
    in_specs = tuple(
        PartitionSpec() if n in SHARED else PartitionSpec("core") for n in in_names
    ) + (PartitionSpec("core"),) * len(out_names)
    sharded = jax.jit(
        shard_map(
            _body,
            mesh=mesh,
            in_specs=in_specs,
            out_specs=(PartitionSpec("core"),) * len(out_names),
            check_rep=False,
        ),
        keep_unused=True,
    )
    zeros = [
        np.zeros((N_CORES * a.shape[0], *a.shape[1:]), a.dtype) for a in out_avals
    ]

    def make_args(in_maps, device=False):
        arrs = []
        for n in in_names:
            if n in SHARED:
                arrs.append(np.asarray(in_maps[0][n]))
            else:
                arrs.append(
                    np.concatenate([np.asarray(m[n]) for m in in_maps], axis=0)
                )
        arrs += list(zeros)
        if device:
            from jax.sharding import NamedSharding

            for i, n in enumerate(in_names):
                sh = NamedSharding(
                    mesh, PartitionSpec() if n in SHARED else PartitionSpec("core")
                )
                arrs[i] = jax.device_put(arrs[i], sh)
            sh = NamedSharding(mesh, PartitionSpec("core"))
            for i in range(len(in_names), len(arrs)):
                arrs[i] = jax.device_put(arrs[i], sh)
        return arrs

    def run(in_maps):
        outs = sharded(*make_args(in_maps))
        res = np.asarray(outs[out_names.index("out")])
        return res.reshape(N_CORES, S, D)

    run.sharded = sharded
    run.make_args = make_args
    run.out_index = out_names.index("out")
    return run


def _prepare(inputs):
    """Host-side prep: mask plan + per-core input maps."""
    x = np.asarray(inputs["x"], np.float32)
    mask = np.asarray(inputs["mask"], bool)[0, 0]
    compress = np.asarray(inputs["compress_neurons"], np.float32)
    rQ = np.asarray(inputs["router_Q"], np.float32)
    rK = np.asarray(inputs["router_K"], np.float32)
    rV = np.asarray(inputs["router_V"], np.float32)
    wQ = np.asarray(inputs["wQ"], np.float32)
    wK = np.asarray(inputs["wK"], np.float32)
    wV = np.asarray(inputs["wV"], np.float32)
    wO = np.asarray(inputs["wO"], np.float32)

    plan, mtiles = _mask_plan(mask)
    nt = len(mtiles)

    # host-side shared prep
    import ml_dtypes

    bf = ml_dtypes.bfloat16
    cflat = np.ascontiguousarray(
        compress.transpose(1, 2, 0).reshape(8, 128, RANK * NCMP)
    ).astype(bf)  # [D, R, NC] -> d-tiles
    routersT = np.ascontiguousarray(
        np.stack([rQ, rK, rV]).transpose(2, 0, 1).reshape(8, 128, 48)
    ).astype(bf)
    wqT = wQ.T * np.float32(1.0 / np.sqrt(DH))  # fold 1/sqrt(dh) into Q
    wkT, wvT = wK.T, wV.T
    wOT = np.ascontiguousarray(wO.T).astype(bf)  # [D, E]

    in_maps = []
    for c in range(N_CORES):
        b, q4 = divmod(c, 4)
        hs = slice(HPC * q4 * DH, HPC * q4 * DH + HPC * DH)
        m = {
            "xT": np.ascontiguousarray(x[b, q4 * SHARD : (q4 + 1) * SHARD, :].T)
            .reshape(8, 128, SHARD)
            .astype(bf),
            "cflat": cflat,
            "routersT": routersT,
            "wqkvT": np.ascontiguousarray(
                np.stack([wqT[:, hs], wkT[:, hs], wvT[:, hs]])
            ).astype(bf),
            "wOT": np.ascontiguousarray(wOT[hs, :]).reshape(2, 128, D),
        }
        if nt:
            m["dmask"] = mtiles
        in_maps.append(m)
    return plan, nt, in_maps


def kernel(**inputs):
    plan, nt, in_maps = _prepare(inputs)
    key = (plan, nt)
    if key not in _RUNNERS:
        _RUNNERS[key] = _make_runner(plan, nt)
    res = _RUNNERS[key](in_maps)  # [8, S, D] bf16 partials
    out = np.empty((B, S, D), np.float32)
    for b in range(B):
        out[b] = res[4 * b : 4 * b + 4].astype(np.float32).sum(axis=0)
    return out


# revision 60
# speedup vs baseline: 1.0802x; 1.0802x over previous
"""MoE-routed low-rank attention (nn_NeuronCircuit_28930899706061) on 8 TRN2 cores.

Sharding: core c in 0..7 -> batch b = c//4, token-shard q4 = c%4 (512 tokens)
for the compress/routing phase; head group heads [4*q4, 4*q4+4) of batch b for
the attention phase. h^T tensors for all 3 routers are all-gathered in ONE
collective within each 4-core batch group; each core emits a partial [S, D]
output (its 4 heads' contribution through wO) and the host sums the 4 partials
per batch.

Attention computes scores TRANSPOSED ([k, q] with k on partitions) so the
softmax-weighted AV needs no P-transpose: AV contracts k on partitions
directly, and the softmax denominator comes for free from a ones-column
appended to V (row 64/63 of the AV PSUM accumulator). Normalization is a
PE outer-product broadcast of 1/z plus one DVE multiply per head.

All weight transposes are done host-side so every device DMA is contiguous.
"""

import os

import numpy as np

# ablation knobs (default = fast path); set to "0" to fall back
# PE transpose-accum combine: correct in CoreSim but WRONG on real HW (bf16
# matmul outputs cannot accumulate in PSUM banks there) -- keep off
KNOB_TA = os.environ.get("KNOB_TA", "0") == "1"
# 0: K=64 per-head wO (no partition move, validated); 1: K=64-lhsT PE shift
# (broken on HW); 2: K=128 baseline-shape PE shift
KNOB_SHIFT = int(os.environ.get("KNOB_SHIFT", "1"))
# affine_select with channel_multiplier=-1: accepted by CoreSim but silently
# wrong on real HW ucode -- default to a multiplicative triangle mask instead
KNOB_AFF = os.environ.get("KNOB_AFF", "0") == "1"
KNOB_PB = os.environ.get("KNOB_PB", "0") == "1"  # gpsimd partition_broadcast
KNOB_STOP = os.environ.get("KNOB_STOP", "")  # "p1": stop after gather
KNOB_TAILX = os.environ.get("KNOB_TAILX", "")  # norec|nobc|nonorm ablations

B, S, D, H, RANK, NCMP = 2, 2048, 1024, 16, 128, 16
DH = D // H  # 64
N_CORES = 8
SHARD = S // 4  # 512 tokens per core in phase 1
HPC = 4  # heads per core
QT_TILES = S // 128  # 16 q tiles
KT_TILES = S // 128  # 16 k tiles

_RUNNERS: dict = {}


def _split_multi_waits(nc, mybir):
    """This toolchain's walrus rejects any instruction carrying >1 sync wait
    ("Too many sync wait commands"); hoist excess waits onto same-engine nops
    inserted immediately before the instruction."""
    cnt = 0
    for f in nc.m.functions:
        for blk in f.blocks:
            il = blk.instructions
            out = []
            changed = False
            for inst in il:
                si = inst.sync_info
                waits = list(si.on_wait or []) if si else []
                if len(waits) > 1:
                    for w in waits[:-1]:
                        cnt += 1
                        nop = mybir.InstNoOp(
                            name=f"wsplit-{cnt}",
                            engine=inst.engine,
                            sync_info=mybir.SyncInfo(on_wait=[w], on_update=[]),
                        )
                        nc.register_instruction(nop)
                        out.append(nop)
                    inst.sync_info = mybir.SyncInfo(
                        on_wait=[waits[-1]], on_update=list(si.on_update or [])
                    )
                    changed = True
                out.append(inst)
            if changed:
                il[:] = out


def _make_tc_class(tile, mybir):
    class TC(tile.TileContext):
        def __exit__(self, *exc):
            ret = super().__exit__(*exc)
            if exc[0] is None:
                _split_multi_waits(self.nc, mybir)
            return ret

    return TC


_TRI = np.tril(np.ones((128, 128), dtype=bool))  # keep k<=q in [k,q] layout is triu


def _mask_plan(maskb):
    """Per q-tile: (nkt, diag_kt, customs) in TRANSPOSED [k, q] tile terms.

    nkt: number of live 128-wide k tiles; diag_kt: k-tile index that is
    exactly the causal triangle (k<=q kept), or None; customs: list of
    (kt, mask_tile_idx) for other partial tiles.  Returns (plan, tiles)
    with tiles a [nt, 128, 128] f32 array of ADDITIVE masks in [k, q]
    orientation.
    """
    tiles = []
    tile_ids = {}
    plan = []
    for qi in range(QT_TILES):
        rows = maskb[qi * 128 : (qi + 1) * 128]  # [128 q, S k]
        nkt = 0
        for kt in range(KT_TILES):
            if rows[:, kt * 128 : (kt + 1) * 128].any():
                nkt = kt + 1
        diag_kt = None
        customs = []
        for kt in range(nkt):
            sub = rows[:, kt * 128 : (kt + 1) * 128]  # [q, k]
            if sub.all():
                continue
            subT = sub.T  # [k, q]
            if diag_kt is None and (subT == _TRI.T).all():
                # exactly "keep k <= q within tile"
                diag_kt = kt
                continue
            add = np.where(subT, np.float32(0), np.float32(-1e30))
            key = add.tobytes()
            if key not in tile_ids:
                tile_ids[key] = len(tiles)
                tiles.append(add)
            customs.append((kt, tile_ids[key]))
        plan.append((nkt, diag_kt, tuple(customs)))
    nt = len(tiles)
    tiles_arr = (
        np.stack(tiles).astype(np.float32)
        if nt
        else np.zeros((0, 128, 128), np.float32)
    )
    return tuple(plan), tiles_arr


def _bcast_mid(bass, ap, n):
    """[P, K] AP -> [P, n, K] AP with a step-0 middle dim (free broadcast)."""
    dims = [list(x) for x in ap.ap]
    return bass.AP(
        tensor=ap.tensor, offset=ap.offset, ap=[dims[0], [0, n]] + dims[1:]
    )


def _build(plan, nt, repeat=1):
    import concourse.bass as bass
    import concourse.mybir as mybir
    import concourse.tile as tile
    from concourse.bass import ts
    from concourse.masks import make_identity, make_upper_triangular

    f32 = mybir.dt.float32
    bf16 = mybir.dt.bfloat16
    Exp = mybir.ActivationFunctionType.Exp
    TC = _make_tc_class(tile, mybir)

    nc = bass.Bass(num_devices=N_CORES)
    xT_d = nc.dram_tensor("xT", [8, 128, SHARD], bf16, kind="ExternalInput")
    cflat_d = nc.dram_tensor("cflat", [8, 128, RANK * NCMP], bf16, kind="ExternalInput")
    routersT_d = nc.dram_tensor("routersT", [8, 128, 48], bf16, kind="ExternalInput")
    wqkvT_d = nc.dram_tensor("wqkvT", [3, 128, HPC * DH], bf16, kind="ExternalInput")
    wOT_d = nc.dram_tensor("wOT", [2, 128, D], bf16, kind="ExternalInput")
    dmask_d = (
        nc.dram_tensor("dmask", [nt, 128, 128], f32, kind="ExternalInput")
        if nt
        else None
    )
    out_d = nc.dram_tensor("out", [S, D], bf16, kind="ExternalOutput")

    groups = [[0, 1, 2, 3], [4, 5, 6, 7]]
    NT4 = SHARD // 128  # 4 s-tiles per core in phase 1

    with TC(nc) as tc:
      for _rep in range(repeat):
        with (
            tc.tile_pool(name="sb", bufs=1) as sbp,
            tc.tile_pool(name="wk2", bufs=2) as wk2,
            tc.tile_pool(name="wk4", bufs=4) as wk4,
            tc.tile_pool(name="dram", bufs=1, space="DRAM") as dramp,
        ):
            ident16 = sbp.tile([128, 128], bf16)
            make_identity(nc, ident16)
            # multiplicative causal mask for [k, q] diagonal tiles:
            # 1.0 where k <= q, 0.0 where k > q
            triu = sbp.tile([128, 128], bf16)
            make_upper_triangular(nc, triu[:], val=1.0, diag=True)
            if not KNOB_TA:
                identf = sbp.tile([128, 128], f32)
                make_identity(nc, identf)
            ones64 = sbp.tile([128, 64], bf16)
            nc.vector.memset(ones64[:], 1.0)
            cbias = sbp.tile([128, 1], f32)
            nc.vector.memset(cbias[:], -20.0)
            cc_in = dramp.tile([128, 3 * SHARD], bf16, name="cc_in")
            cc_out = dramp.tile([4, 128, 3 * SHARD], bf16, name="cc_out")

            # phase-1 inputs first on the DMA queue (they gate the pipeline)
            xt = sbp.tile([128, 8, SHARD], bf16)
            rt = sbp.tile([128, 8, 48], bf16)
            cfl = sbp.tile([128, 8, RANK * NCMP], bf16)
            nc.sync.dma_start(xt[:], xT_d[:].rearrange("d p s -> p d s"))
            nc.sync.dma_start(rt[:], routersT_d[:].rearrange("d p s -> p d s"))
            nc.sync.dma_start(
                cfl[:, :, ts(0, 512)],
                cflat_d[:].rearrange("d p s -> p d s")[:, :, ts(0, 512)],
            )
            # attention-side constant loads
            wq = sbp.tile([128, 3, HPC * DH], bf16)
            for r in range(3):
                nc.sync.dma_start(wq[:, r, :], wqkvT_d[r])
            if KNOB_SHIFT:
                wo = sbp.tile([128, 2, D], bf16)
                for k in range(2):
                    nc.sync.dma_start(wo[:, k, :], wOT_d[k])
            else:
                # per-head wO rows on partitions 0..63 (K=64 accumulation)
                wo4 = sbp.tile([64, HPC, D], bf16)
                for k in range(2):
                    nc.sync.dma_start(
                        wo4[:, 2 * k : 2 * k + 2, :],
                        wOT_d[k].rearrange("(h p) e -> p h e", h=2),
                    )
            # remaining compress-weight chunks
            for ch in range(1, 4):
                for dk in range(8):
                    nc.sync.dma_start(
                        cfl[:, dk, ts(ch, 512)], cflat_d[dk][:, ts(ch, 512)]
                    )
            if nt:
                masks = sbp.tile([128, nt, 128], f32)
                for t in range(nt):
                    nc.sync.dma_start(masks[:, t, :], dmask_d[t])
            # V' tile: per (k-tile, head): [V | 1]; the ones column makes
            # row 64 of the AV accumulator the softmax denominator z
            vV = sbp.tile([128, KT_TILES, HPC, 65], bf16)
            nc.gpsimd.memset(vV[:, :, :, 64:65], 1.0)
            if KNOB_SHIFT == 2:
                # persistent double-buffered odd-head scratch; rows 64..127
                # stay zero so the K=128 ident-shift reads clean zeros there
                aoddp = sbp.tile([128, 2, 2, 128], bf16)
                nc.vector.memset(aoddp[:], 0.0)
            if KNOB_PB:
                from concourse.library_config import attn as _attnlib

                nc.gpsimd.load_library(_attnlib)

            with (
                tc.tile_pool(name="p1y", bufs=2, space="PSUM") as p1y,
                tc.tile_pool(name="p1w", bufs=1, space="PSUM") as p1w,
                tc.tile_pool(name="p1t", bufs=2, space="PSUM") as p1t,
            ):
                # ------------- phase 1: routing + compress -------------

                ys = [
                    sbp.tile([128, RANK * NCMP], bf16, name=f"y{t}")
                    for t in range(NT4)
                ]
                w3nb = sbp.tile([128, NT4, 48], bf16)

                def emit_combine(t):
                    """Weighted neuron-sum + transpose + gather-staging for one
                    finished s-tile.  Deferred one tile so the PE never stalls
                    on the DVE multiply."""
                    for r in range(3):
                        yv = ys[t][:].rearrange("p (r n) -> p r n", n=NCMP)
                        tmp = wk2.tile([128, RANK * NCMP], bf16, tag="tmp")
                        tv = tmp[:].rearrange("p (r n) -> p r n", n=NCMP)
                        wb = _bcast_mid(bass, w3nb[:, t, ts(r, 16)], RANK)
                        if r == 2:
                            # gpsimd TensorTensor (standard library) relieves
                            # the DVE, which carries the grouped reduces
                            nc.gpsimd.tensor_mul(tv, yv, wb)
                        else:
                            nc.vector.tensor_mul(tv, yv, wb)
                        if not KNOB_TA:
                            # HW-validated: DVE grouped reduce + f32 transpose
                            hT = p1t.tile([128, 128], f32, tag="hT")
                            h = wk4.tile([128, RANK], f32, tag="h")
                            nc.vector.tensor_reduce(
                                h[:], tv, axis=mybir.AxisListType.X,
                                op=mybir.AluOpType.add,
                            )
                            nc.tensor.matmul(
                                hT[:], h[:], identf[:], is_transpose=True,
                                start=True, stop=True, skip_group_check=True,
                            )
                        else:
                            # PE-fused: 16 accumulating bf16 transpose passes
                            # (CoreSim-only -- bf16 PSUM accumulation is not
                            # trustworthy on real HW)
                            hT = p1t.tile([128, 128], bf16, tag="hT")
                            for n in range(NCMP):
                                nc.tensor.matmul(
                                    hT[:], tv[:, :, n], ident16[:],
                                    is_transpose=True,
                                    start=(n == 0), stop=(n == NCMP - 1),
                                    skip_group_check=True,
                                )
                        hTs = wk4.tile([128, 128], bf16, tag="hTs")
                        nc.scalar.copy(hTs[:], hT[:])
                        nc.sync.dma_start(
                            cc_in[:, r * SHARD + t * 128 : r * SHARD + (t + 1) * 128],
                            hTs[:],
                        )

                for t in range(NT4):
                    w3ps = p1w.tile([128, 48], f32, tag="w3ps")
                    for dk in range(8):
                        nc.tensor.matmul(
                            w3ps[:], xt[:, dk, ts(t, 128)], rt[:, dk, :],
                            start=(dk == 0), stop=(dk == 7),
                        )
                    for r in range(3):
                        # no max-subtraction: router logits are O(1)
                        e3 = wk4.tile([128, 16], f32, tag="e3")
                        z3 = wk4.tile([128, 1], f32, tag="z3")
                        nc.scalar.activation(
                            e3[:], w3ps[:, ts(r, 16)], Exp,
                            bias=0.0, scale=1.0, accum_out=z3[:],
                        )
                        rz3 = wk4.tile([128, 1], f32, tag="rz3")
                        nc.vector.reciprocal(rz3[:], z3[:])
                        nc.vector.tensor_scalar_mul(
                            w3nb[:, t, ts(r, 16)], e3[:], rz3[:]
                        )

                    # y[s, (r, n)] = x @ C  (r outer, n inner), 2 chunks/bank-pair
                    for cp in range(2):
                        yp = p1y.tile([128, 2, 512], f32, tag="yp")
                        for c2 in range(2):
                            ch = cp * 2 + c2
                            for dk in range(8):
                                nc.tensor.matmul(
                                    yp[:, c2, :], xt[:, dk, ts(t, 128)],
                                    cfl[:, dk, ts(ch, 512)],
                                    start=(dk == 0), stop=(dk == 7),
                                )
                        nc.scalar.copy(ys[t][:, ts(cp, 1024)], yp[:])
                    if t > 0:
                        emit_combine(t - 1)
                emit_combine(NT4 - 1)
                # single merged AllGather for all 3 routers
                nc.gpsimd.collective_compute(
                    "AllGather", mybir.AluOpType.bypass,
                    replica_groups=groups,
                    ins=[cc_in[:]], outs=[cc_out[:]],
                )

            if KNOB_STOP == "p1":
                # timing bisect: consume cc_out, skip expand+attention
                dumb = sbp.tile([128, 512], bf16)
                nc.sync.dma_start(
                    dumb[:],
                    bass.AP(
                        tensor=cc_out.tensor, offset=cc_out.offset,
                        ap=[[3 * SHARD, 128], [1, 512]],
                    ),
                )
                nc.sync.dma_start(out_d[0:128, 0:512], dumb[:])
                continue

            # ------------- expand: Q^T, K^T, V' -------------
            hxt = sbp.tile([128, 3, S], bf16)
            for r in range(3):
                srcap = bass.AP(
                    tensor=cc_out.tensor,
                    offset=cc_out.offset + r * SHARD,
                    ap=[[3 * SHARD, 128], [128 * 3 * SHARD, 4], [1, SHARD]],
                )
                nc.sync.dma_start(hxt[:, r, :], srcap)

            QT = sbp.tile([128, 2, S], bf16)
            KT = sbp.tile([128, 2, S], bf16)
            with tc.tile_pool(name="qkps", bufs=3, space="PSUM") as qkps:
                for r, dst in ((0, QT), (1, KT)):
                    for pair in range(2):
                        for ch in range(4):
                            ps = qkps.tile([128, 512], f32, tag="qk")
                            nc.tensor.matmul(
                                ps[:], wq[:, r, ts(pair, 128)],
                                hxt[:, r, ts(ch, 512)],
                                start=True, stop=True,
                            )
                            nc.vector.tensor_copy(dst[:, pair, ts(ch, 512)], ps[:])
                for st in range(KT_TILES):
                    vps = qkps.tile([128, HPC * DH], f32, tag="qk")
                    nc.tensor.matmul(
                        vps[:], hxt[:, 2, ts(st, 128)], wq[:, 2, :],
                        start=True, stop=True,
                    )
                    nc.vector.tensor_copy(
                        vV[:, st, :, 0:64],
                        vps[:].rearrange("p (h d) -> p h d", d=DH),
                    )

            # ------------- attention + wO -------------
            with (
                tc.tile_pool(name="stp", bufs=2, space="PSUM") as stp,
                tc.tile_pool(name="avp", bufs=2, space="PSUM") as avp,
                tc.tile_pool(name="bcp", bufs=1, space="PSUM") as bcp,
                tc.tile_pool(name="opp", bufs=1, space="PSUM") as opp,
                tc.tile_pool(name="wkp", bufs=3) as wkp,
            ):

                def emit_norm(qi, av):
                    """Normalize a finished q-tile into a pair-stacked aoT.
                    Deferred into the next q-tile's issue window so the queues
                    never stall on the AV->recip->bc->mul chain."""
                    rzt = wk2.tile([128, HPC, 128], bf16, tag="rzt")
                    # 1/z for all 4 heads in one op (z rows on partition 64).
                    # The extra copy matters: a non-DVE engine waiting directly
                    # on a Reciprocal's completion sem stalls ~600us on HW
                    # (event-accel pathology); the PE waits on the copy instead.
                    if KNOB_TAILX in ("norec", "nonorm"):
                        nc.vector.memset(rzt[64:65, :, :], 1.0)
                    else:
                        rzr = wk4.tile([128, HPC, 128], bf16, tag="rzr")
                        with nc.allow_low_precision(
                            reason="1/z in bf16: 0.4% relative, well under tol"
                        ):
                            nc.vector.reciprocal(rzr[64:65, :, :], av[64:65, :, :])
                        nc.vector.tensor_copy(rzt[64:65, :, :], rzr[64:65, :, :])
                    # broadcast 1/z to 64 rows (PE outer product, or gpsimd
                    # partition_broadcast), then normalize each head's [64, q]
                    # tile into bf16.  The PSUM bank also hosts the odd-head
                    # partition-shift staging on partitions 64..127.
                    bsh = bcp.tile([128, HPC, 128], f32, tag="bc")
                    bcs = wk2.tile([64, HPC, 128], bf16, tag="bcs")
                    if KNOB_TAILX in ("nobc", "nonorm"):
                        nc.vector.memset(bcs[:], 1.0)
                    elif KNOB_PB:
                        nc.gpsimd.partition_broadcast(
                            bcs[:], rzt[64:65, :, :], channels=64
                        )
                    else:
                        for h in range(HPC):
                            nc.tensor.matmul(
                                bsh[0:64, h, :], ones64[64:65, :],
                                rzt[64:65, h, :],
                                start=True, stop=True, skip_group_check=True,
                            )
                        nc.vector.tensor_copy(bcs[:], bsh[0:64, :, :])
                    # pair-stacked aoT for a K=128 wO: even heads' normalized
                    # tiles are written in place (partitions 0..63); odd heads
                    # go to a scratch tile and a PE identity-matmul moves both
                    # to partitions 64..127 (compute engines are lane-locked,
                    # but the PE can write any PSUM partition range)
                    if KNOB_SHIFT:
                        aoT = wk2.tile([128, 2, 128], bf16, tag="aoT")
                        if KNOB_SHIFT == 2:
                            aodd = aoddp[:, qi % 2, :, :]
                        else:
                            aodd_t = wk2.tile([64, 2, 128], bf16, tag="aodd")
                            aodd = aodd_t[:]
                        for pair in range(2):
                            nc.vector.tensor_mul(
                                aodd[0:64, pair, :], av[0:64, 2 * pair + 1, :],
                                bcs[:, 2 * pair + 1, :],
                            )
                        for pair in range(2):
                            if KNOB_SHIFT == 2:
                                nc.tensor.matmul(
                                    bsh[64:128, pair, :], ident16[:, 0:64],
                                    aodd[:, pair, :],
                                    start=True, stop=True,
                                    skip_group_check=True,
                                )
                            else:
                                nc.tensor.matmul(
                                    bsh[64:128, pair, :], ident16[0:64, 0:64],
                                    aodd[0:64, pair, :],
                                    start=True, stop=True,
                                    skip_group_check=True,
                                )
                        nc.scalar.copy(aoT[64:128, :, :], bsh[64:128, 0:2, :])
                        for pair in range(2):
                            nc.vector.tensor_mul(
                                aoT[0:64, pair, :], av[0:64, 2 * pair, :],
                                bcs[:, 2 * pair, :],
                            )
                    else:
                        aoT = wk2.tile([64, HPC, 128], bf16, tag="aoT")
                        for h in range(HPC):
                            nc.vector.tensor_mul(
                                aoT[:, h, :], av[0:64, h, :], bcs[:, h, :]
                            )
                    return aoT

                def emit_wo(qi, aoT):
                    for e in range(2):
                        ops = opp.tile([128, 512], f32, tag="o")
                        if KNOB_SHIFT:
                            for pair in range(2):
                                nc.tensor.matmul(
                                    ops[:], aoT[:, pair, :],
                                    wo[:, pair, ts(e, 512)],
                                    start=(pair == 0), stop=(pair == 1),
                                    skip_group_check=True,
                                )
                        else:
                            for h in range(HPC):
                                nc.tensor.matmul(
                                    ops[:], aoT[:, h, :], wo4[:, h, ts(e, 512)],
                                    start=(h == 0), stop=(h == 3),
                                    skip_group_check=True,
                                )
                        osb = wk2.tile([128, 512], bf16, tag="osb")
                        nc.vector.tensor_copy(osb[:], ops[:])
                        nc.sync.dma_start(out_d[ts(qi, 128), ts(e, 512)], osb[:])

                def emit_scores(qi, h, nkt, diag_kt, customs):
                    """QK^T + exp for one (q-tile, head); returns the P tile."""
                    pair, doff = h // 2, (h % 2) * 64
                    p_sb = wkp.tile([128, KT_TILES, 128], bf16, tag="p")
                    for kb in range(0, nkt, 8):
                        kw = min(8, nkt - kb)
                        st_ps = stp.tile([128, 8, 128], f32, tag="st")
                        for j in range(kw):
                            nc.tensor.matmul(
                                st_ps[:, j, :],
                                KT[doff : doff + 64, pair, ts(kb + j, 128)],
                                QT[doff : doff + 64, pair, ts(qi, 128)],
                                start=True, stop=True,
                                skip_group_check=True,
                            )
                        for kt, mi in customs:
                            if kb <= kt < kb + kw:
                                nc.vector.tensor_add(
                                    st_ps[:, kt - kb, :],
                                    st_ps[:, kt - kb, :],
                                    masks[:, mi, :],
                                )
                        # exp(score - 20); the shift cancels in the
                        # softmax normalization
                        nc.scalar.activation(
                            p_sb[:, kb : kb + kw, :], st_ps[:, 0:kw, :],
                            Exp, bias=cbias[:], scale=1.0,
                        )
                        if diag_kt is not None and kb <= diag_kt < kb + kw:
                            # zero P where k > q
                            if KNOB_AFF:
                                nc.gpsimd.affine_select(
                                    p_sb[:, diag_kt, :], p_sb[:, diag_kt, :],
                                    pattern=[[1, 128]],
                                    compare_op=mybir.AluOpType.is_ge,
                                    fill=0.0,
                                    base=0,
                                    channel_multiplier=-1,
                                )
                            else:
                                # gpsimd standard-library TensorTensor: the
                                # Pool queue is idle during attention
                                nc.gpsimd.tensor_mul(
                                    p_sb[:, diag_kt, :], p_sb[:, diag_kt, :],
                                    triu[:],
                                )
                    return p_sb

                def emit_av(av, h, nkt, p_sb):
                    for kt in range(nkt):
                        nc.tensor.matmul(
                            av[:, h, :],
                            vV[:, kt, h, :],
                            p_sb[:, kt, :],
                            start=(kt == 0), stop=(kt == nkt - 1),
                            skip_group_check=True,
                        )

                # slot-based software pipeline over the 64 (q-tile, head)
                # iterations: AV lags its scores by 2 slots (hides the Act exp
                # latency), each tile's normalize chain runs one slot after its
                # last AV, and its wO two slots later -- so no engine queue
                # ever sits on a cross-engine dependency.
                slot = 0
                due = []  # (due_slot, fn), FIFO per slot

                def sched(delay, fn):
                    due.append((slot + delay, fn))

                def run_due():
                    i = 0
                    while i < len(due):
                        s, fn = due[i]
                        if s <= slot:
                            due.pop(i)
                            fn()
                        else:
                            i += 1

                def make_av(av, h, nkt, p_sb):
                    return lambda: emit_av(av, h, nkt, p_sb)

                def make_norm(qi, av, box):
                    def fn():
                        box.append(emit_norm(qi, av))

                    return fn

                def make_wo(qi, box):
                    return lambda: emit_wo(qi, box[0])

                # alternate big/small tiles to smooth the engine mix
                _ord = []
                _big = sorted(range(QT_TILES), key=lambda q: -plan[q][0])
                for _i in range(QT_TILES // 2):
                    _ord += [_big[_i], _big[QT_TILES - 1 - _i]]
                for qi in _ord:
                    nkt, diag_kt, customs = plan[qi]
                    av = avp.tile([65, HPC, 128], f32, tag="av")
                    for h in range(HPC):
                        p_sb = emit_scores(qi, h, nkt, diag_kt, customs)
                        if KNOB_STOP != "qk":
                            sched(2, make_av(av, h, nkt, p_sb))
                        if h == 3 and not KNOB_STOP:
                            box = []
                            sched(3, make_norm(qi, av, box))
                            sched(5, make_wo(qi, box))
                        slot += 1
                        run_due()
                # flush
                slot += 16
                run_due()
                if KNOB_STOP in ("qk", "av"):
                    dumb2 = sbp.tile([128, 512], bf16)
                    nc.vector.tensor_copy(dumb2[:], QT[:, 0, 0:512])
                    nc.sync.dma_start(out_d[0:128, 0:512], dumb2[:])

    return nc


def _make_runner(plan, nt, repeat=1):
    """Compile the graph once and return fn(in_maps) -> list of out arrays."""
    import jax
    import numpy as np
    from jax.sharding import Mesh, PartitionSpec
    from jax.experimental.shard_map import shard_map
    import concourse.bass2jax as bass2jax
    import concourse.mybir as mybir

    nc = _build(plan, nt, repeat=repeat)
    bass2jax.install_neuronx_cc_hook()

    partition_name = nc.partition_id_tensor.name if nc.partition_id_tensor else None
    in_names, out_names, out_avals = [], [], []
    for alloc in nc.m.functions[0].allocations:
        if not isinstance(alloc, mybir.MemoryLocationSet):
            continue
        name = alloc.memorylocations[0].name
        if alloc.kind == "ExternalInput":
            if name != partition_name:
                in_names.append(name)
        elif alloc.kind == "ExternalOutput":
            out_names.append(name)
            out_avals.append(
                jax.core.ShapedArray(
                    tuple(alloc.tensor_shape), mybir.dt.np(alloc.dtype)
                )
            )
    all_names = in_names + out_names
    if partition_name is not None:
        all_names = all_names + [partition_name]

    def _body(*args):
        operands = list(args)
        if partition_name is not None:
            operands.append(bass2jax.partition_id_tensor())
        outs = bass2jax._bass_exec_p.bind(
            *operands,
            out_avals=tuple(out_avals),
            in_names=tuple(all_names),
            out_names=tuple(out_names),
            lowering_input_output_aliases=(),
            sim_require_finite=True,
            sim_require_nnan=True,
            nc=nc,
        )
        return tuple(outs)

    devices = jax.devices()[:N_CORES]
    mesh = Mesh(np.asarray(devices), ("core",))
    SHARED = {"cflat", "routersT", "dmask"}
    in_specs = tuple(
        PartitionSpec() if n in SHARED else PartitionSpec("core") for n in in_names
    ) + (PartitionSpec("core"),) * len(out_names)
    sharded = jax.jit(
        shard_map(
            _body,
            mesh=mesh,
            in_specs=in_specs,
            out_specs=(PartitionSpec("core"),) * len(out_names),
            check_rep=False,
        ),
        keep_unused=True,
    )
    zeros = [
        np.zeros((N_CORES * a.shape[0], *a.shape[1:]), a.dtype) for a in out_avals
    ]

    def make_args(in_maps, device=False):
        arrs = []
        for n in in_names:
            if n in SHARED:
                arrs.append(np.asarray(in_maps[0][n]))
            else:
                arrs.append(
                    np.concatenate([np.asarray(m[n]) for m in in_maps], axis=0)
                )
        arrs += list(zeros)
        if device:
            from jax.sharding import NamedSharding

            for i, n in enumerate(in_names):
                sh = NamedSharding(
                    mesh, PartitionSpec() if n in SHARED else PartitionSpec("core")
                )
                arrs[i] = jax.device_put(arrs[i], sh)
            sh = NamedSharding(mesh, PartitionSpec("core"))
            for i in range(len(in_names), len(arrs)):
                arrs[i] = jax.device_put(arrs[i], sh)
        return arrs

    def run(in_maps):
        outs = sharded(*make_args(in_maps))
        res = np.asarray(outs[out_names.index("out")])
        return res.reshape(N_CORES, S, D)

    run.sharded = sharded
    run.make_args = make_args
    run.out_index = out_names.index("out")
    return run


def _prepare(inputs):
    """Host-side prep: mask plan + per-core input maps."""
    x = np.asarray(inputs["x"], np.float32)
    mask = np.asarray(inputs["mask"], bool)[0, 0]
    compress = np.asarray(inputs["compress_neurons"], np.float32)
    rQ = np.asarray(inputs["router_Q"], np.float32)
    rK = np.asarray(inputs["router_K"], np.float32)
    rV = np.asarray(inputs["router_V"], np.float32)
    wQ = np.asarray(inputs["wQ"], np.float32)
    wK = np.asarray(inputs["wK"], np.float32)
    wV = np.asarray(inputs["wV"], np.float32)
    wO = np.asarray(inputs["wO"], np.float32)

    plan, mtiles = _mask_plan(mask)
    nt = len(mtiles)

    # host-side shared prep
    import ml_dtypes

    bf = ml_dtypes.bfloat16
    cflat = np.ascontiguousarray(
        compress.transpose(1, 2, 0).reshape(8, 128, RANK * NCMP)
    ).astype(bf)  # [D, R, NC] -> d-tiles
    routersT = np.ascontiguousarray(
        np.stack([rQ, rK, rV]).transpose(2, 0, 1).reshape(8, 128, 48)
    ).astype(bf)
    wqT = wQ.T * np.float32(1.0 / np.sqrt(DH))  # fold 1/sqrt(dh) into Q
    wkT, wvT = wK.T, wV.T
    wOT = np.ascontiguousarray(wO.T).astype(bf)  # [D, E]

    in_maps = []
    for c in range(N_CORES):
        b, q4 = divmod(c, 4)
        hs = slice(HPC * q4 * DH, HPC * q4 * DH + HPC * DH)
        m = {
            "xT": np.ascontiguousarray(x[b, q4 * SHARD : (q4 + 1) * SHARD, :].T)
            .reshape(8, 128, SHARD)
            .astype(bf),
            "cflat": cflat,
            "routersT": routersT,
            "wqkvT": np.ascontiguousarray(
                np.stack([wqT[:, hs], wkT[:, hs], wvT[:, hs]])
            ).astype(bf),
            "wOT": np.ascontiguousarray(wOT[hs, :]).reshape(2, 128, D),
        }
        if nt:
            m["dmask"] = mtiles
        in_maps.append(m)
    return plan, nt, in_maps


def kernel(**inputs):
    plan, nt, in_maps = _prepare(inputs)
    key = (plan, nt)
    if key not in _RUNNERS:
        _RUNNERS[key] = _make_runner(plan, nt)
    res = _RUNNERS[key](in_maps)  # [8, S, D] bf16 partials
    out = np.empty((B, S, D), np.float32)
    for b in range(B):
        out[b] = res[4 * b : 4 * b + 4].astype(np.float32).sum(axis=0)
    return out


# revision 65
# speedup vs baseline: 32.8481x; 30.4098x over previous
"""MoE-routed low-rank attention (nn_NeuronCircuit_28930899706061) on 8 TRN2 cores.

Sharding: core c in 0..7 -> batch b = c//4, token-shard q4 = c%4 (512 tokens)
for the compress/routing phase; head group heads [4*q4, 4*q4+4) of batch b for
the attention phase. h^T tensors for all 3 routers are all-gathered in ONE
collective within each 4-core batch group; each core emits a partial [S, D]
output (its 4 heads' contribution through wO) and the host sums the 4 partials
per batch.

Attention computes scores TRANSPOSED ([k, q] with k on partitions) so the
softmax-weighted AV needs no P-transpose: AV contracts k on partitions
directly, and the softmax denominator comes for free from a ones-column
appended to V (row 64/63 of the AV PSUM accumulator). Normalization is a
PE outer-product broadcast of 1/z plus one DVE multiply per head.

All weight transposes are done host-side so every device DMA is contiguous.
"""

import os

import numpy as np

# ablation knobs (default = fast path); set to "0" to fall back
# PE transpose-accum combine: correct in CoreSim but WRONG on real HW (bf16
# matmul outputs cannot accumulate in PSUM banks there) -- keep off
KNOB_TA = os.environ.get("KNOB_TA", "0") == "1"
# 0: K=64 per-head wO (no partition move, validated); 1: K=64-lhsT PE shift
# (broken on HW); 2: K=128 baseline-shape PE shift
KNOB_SHIFT = int(os.environ.get("KNOB_SHIFT", "1"))
# affine_select with channel_multiplier=-1: accepted by CoreSim but silently
# wrong on real HW ucode -- default to a multiplicative triangle mask instead
KNOB_AFF = os.environ.get("KNOB_AFF", "0") == "1"
KNOB_PB = os.environ.get("KNOB_PB", "0") == "1"  # gpsimd partition_broadcast
KNOB_STOP = os.environ.get("KNOB_STOP", "")  # "p1": stop after gather
KNOB_TAILX = os.environ.get("KNOB_TAILX", "")  # norec|nobc|nonorm ablations

B, S, D, H, RANK, NCMP = 2, 2048, 1024, 16, 128, 16
DH = D // H  # 64
N_CORES = 8
SHARD = S // 4  # 512 tokens per core in phase 1
HPC = 4  # heads per core
QT_TILES = S // 128  # 16 q tiles
KT_TILES = S // 128  # 16 k tiles

_RUNNERS: dict = {}


def _split_multi_waits(nc, mybir):
    """This toolchain's walrus rejects any instruction carrying >1 sync wait
    ("Too many sync wait commands"); hoist excess waits onto same-engine nops
    inserted immediately before the instruction."""
    cnt = 0
    for f in nc.m.functions:
        for blk in f.blocks:
            il = blk.instructions
            out = []
            changed = False
            for inst in il:
                si = inst.sync_info
                waits = list(si.on_wait or []) if si else []
                if len(waits) > 1:
                    for w in waits[:-1]:
                        cnt += 1
                        nop = mybir.InstNoOp(
                            name=f"wsplit-{cnt}",
                            engine=inst.engine,
                            sync_info=mybir.SyncInfo(on_wait=[w], on_update=[]),
                        )
                        nc.register_instruction(nop)
                        out.append(nop)
                    inst.sync_info = mybir.SyncInfo(
                        on_wait=[waits[-1]], on_update=list(si.on_update or [])
                    )
                    changed = True
                out.append(inst)
            if changed:
                il[:] = out


def _make_tc_class(tile, mybir):
    class TC(tile.TileContext):
        def __exit__(self, *exc):
            ret = super().__exit__(*exc)
            if exc[0] is None:
                _split_multi_waits(self.nc, mybir)
            return ret

    return TC


_TRI = np.tril(np.ones((128, 128), dtype=bool))  # keep k<=q in [k,q] layout is triu


def _mask_plan(maskb):
    """Per q-tile: (nkt, diag_kt, customs) in TRANSPOSED [k, q] tile terms.

    nkt: number of live 128-wide k tiles; diag_kt: k-tile index that is
    exactly the causal triangle (k<=q kept), or None; customs: list of
    (kt, mask_tile_idx) for other partial tiles.  Returns (plan, tiles)
    with tiles a [nt, 128, 128] f32 array of ADDITIVE masks in [k, q]
    orientation.
    """
    tiles = []
    tile_ids = {}
    plan = []
    for qi in range(QT_TILES):
        rows = maskb[qi * 128 : (qi + 1) * 128]  # [128 q, S k]
        nkt = 0
        for kt in range(KT_TILES):
            if rows[:, kt * 128 : (kt + 1) * 128].any():
                nkt = kt + 1
        diag_kt = None
        customs = []
        for kt in range(nkt):
            sub = rows[:, kt * 128 : (kt + 1) * 128]  # [q, k]
            if sub.all():
                continue
            subT = sub.T  # [k, q]
            if diag_kt is None and (subT == _TRI.T).all():
                # exactly "keep k <= q within tile"
                diag_kt = kt
                continue
            add = np.where(subT, np.float32(0), np.float32(-1e30))
            key = add.tobytes()
            if key not in tile_ids:
                tile_ids[key] = len(tiles)
                tiles.append(add)
            customs.append((kt, tile_ids[key]))
        plan.append((nkt, diag_kt, tuple(customs)))
    nt = len(tiles)
    tiles_arr = (
        np.stack(tiles).astype(np.float32)
        if nt
        else np.zeros((0, 128, 128), np.float32)
    )
    return tuple(plan), tiles_arr


def _bcast_mid(bass, ap, n):
    """[P, K] AP -> [P, n, K] AP with a step-0 middle dim (free broadcast)."""
    dims = [list(x) for x in ap.ap]
    return bass.AP(
        tensor=ap.tensor, offset=ap.offset, ap=[dims[0], [0, n]] + dims[1:]
    )


def _build(plan, nt, repeat=1):
    import concourse.bass as bass
    import concourse.mybir as mybir
    import concourse.tile as tile
    from concourse.bass import ts
    from concourse.masks import make_identity, make_upper_triangular

    f32 = mybir.dt.float32
    bf16 = mybir.dt.bfloat16
    Exp = mybir.ActivationFunctionType.Exp
    Ln = mybir.ActivationFunctionType.Ln
    TC = _make_tc_class(tile, mybir)

    nc = bass.Bass(num_devices=N_CORES)
    xT_d = nc.dram_tensor("xT", [8, 128, SHARD], bf16, kind="ExternalInput")
    cflat_d = nc.dram_tensor("cflat", [8, 128, RANK * NCMP], bf16, kind="ExternalInput")
    routersT_d = nc.dram_tensor("routersT", [8, 128, 48], bf16, kind="ExternalInput")
    wqkvT_d = nc.dram_tensor("wqkvT", [3, 128, HPC * DH], bf16, kind="ExternalInput")
    wOT_d = nc.dram_tensor("wOT", [2, 128, D], bf16, kind="ExternalInput")
    dmask_d = (
        nc.dram_tensor("dmask", [nt, 128, 128], f32, kind="ExternalInput")
        if nt
        else None
    )
    out_d = nc.dram_tensor("out", [S, D], bf16, kind="ExternalOutput")

    groups = [[0, 1, 2, 3], [4, 5, 6, 7]]
    NT4 = SHARD // 128  # 4 s-tiles per core in phase 1

    with TC(nc) as tc:
      for _rep in range(repeat):
        with (
            tc.tile_pool(name="sb", bufs=1) as sbp,
            tc.tile_pool(name="wk2", bufs=2) as wk2,
            tc.tile_pool(name="wk4", bufs=4) as wk4,
            tc.tile_pool(name="dram", bufs=1, space="DRAM") as dramp,
        ):
            ident16 = sbp.tile([128, 128], bf16)
            make_identity(nc, ident16)
            # multiplicative causal mask for [k, q] diagonal tiles:
            # 1.0 where k <= q, 0.0 where k > q
            triu = sbp.tile([128, 128], bf16)
            make_upper_triangular(nc, triu[:], val=1.0, diag=True)
            if not KNOB_TA:
                identf = sbp.tile([128, 128], f32)
                make_identity(nc, identf)
            ones64 = sbp.tile([128, 64], bf16)
            nc.vector.memset(ones64[:], 1.0)
            cbias = sbp.tile([128, 1], f32)
            nc.vector.memset(cbias[:], -20.0)
            cc_in = dramp.tile([128, 3 * SHARD], bf16, name="cc_in")
            cc_out = dramp.tile([4, 128, 3 * SHARD], bf16, name="cc_out")

            # phase-1 inputs first on the DMA queue (they gate the pipeline)
            xt = sbp.tile([128, 8, SHARD], bf16)
            rt = sbp.tile([128, 8, 48], bf16)
            cfl = sbp.tile([128, 8, RANK * NCMP], bf16)
            nc.sync.dma_start(xt[:], xT_d[:].rearrange("d p s -> p d s"))
            nc.sync.dma_start(rt[:], routersT_d[:].rearrange("d p s -> p d s"))
            nc.sync.dma_start(
                cfl[:, :, ts(0, 512)],
                cflat_d[:].rearrange("d p s -> p d s")[:, :, ts(0, 512)],
            )
            # attention-side constant loads
            wq = sbp.tile([128, 3, HPC * DH], bf16)
            for r in range(3):
                nc.sync.dma_start(wq[:, r, :], wqkvT_d[r])
            if KNOB_SHIFT:
                wo = sbp.tile([128, 2, D], bf16)
                for k in range(2):
                    nc.sync.dma_start(wo[:, k, :], wOT_d[k])
            else:
                # per-head wO rows on partitions 0..63 (K=64 accumulation)
                wo4 = sbp.tile([64, HPC, D], bf16)
                for k in range(2):
                    nc.sync.dma_start(
                        wo4[:, 2 * k : 2 * k + 2, :],
                        wOT_d[k].rearrange("(h p) e -> p h e", h=2),
                    )
            # remaining compress-weight chunks
            for ch in range(1, 4):
                for dk in range(8):
                    nc.sync.dma_start(
                        cfl[:, dk, ts(ch, 512)], cflat_d[dk][:, ts(ch, 512)]
                    )
            if nt:
                masks = sbp.tile([128, nt, 128], f32)
                for t in range(nt):
                    nc.sync.dma_start(masks[:, t, :], dmask_d[t])
            # V' tile: per (k-tile, head): [V | 1]; the ones column makes
            # row 64 of the AV accumulator the softmax denominator z
            vV = sbp.tile([128, KT_TILES, HPC, 65], bf16)
            nc.gpsimd.memset(vV[:, :, :, 64:65], 1.0)
            if KNOB_SHIFT == 2:
                # persistent double-buffered odd-head scratch; rows 64..127
                # stay zero so the K=128 ident-shift reads clean zeros there
                aoddp = sbp.tile([128, 2, 2, 128], bf16)
                nc.vector.memset(aoddp[:], 0.0)
            if KNOB_PB:
                from concourse.library_config import attn as _attnlib

                nc.gpsimd.load_library(_attnlib)

            with (
                tc.tile_pool(name="p1y", bufs=2, space="PSUM") as p1y,
                tc.tile_pool(name="p1w", bufs=1, space="PSUM") as p1w,
                tc.tile_pool(name="p1t", bufs=2, space="PSUM") as p1t,
            ):
                # ------------- phase 1: routing + compress -------------

                ys = [
                    sbp.tile([128, RANK * NCMP], bf16, name=f"y{t}")
                    for t in range(NT4)
                ]
                w3nb = sbp.tile([128, NT4, 48], bf16)

                def emit_combine(t):
                    """Weighted neuron-sum + transpose + gather-staging for one
                    finished s-tile.  Deferred one tile so the PE never stalls
                    on the DVE multiply."""
                    for r in range(3):
                        yv = ys[t][:].rearrange("p (r n) -> p r n", n=NCMP)
                        tmp = wk2.tile([128, RANK * NCMP], bf16, tag="tmp")
                        tv = tmp[:].rearrange("p (r n) -> p r n", n=NCMP)
                        wb = _bcast_mid(bass, w3nb[:, t, ts(r, 16)], RANK)
                        if r == 2:
                            # gpsimd TensorTensor (standard library) relieves
                            # the DVE, which carries the grouped reduces
                            nc.gpsimd.tensor_mul(tv, yv, wb)
                        else:
                            nc.vector.tensor_mul(tv, yv, wb)
                        if not KNOB_TA:
                            # HW-validated: DVE grouped reduce + f32 transpose
                            hT = p1t.tile([128, 128], f32, tag="hT")
                            h = wk4.tile([128, RANK], f32, tag="h")
                            nc.vector.tensor_reduce(
                                h[:], tv, axis=mybir.AxisListType.X,
                                op=mybir.AluOpType.add,
                            )
                            nc.tensor.matmul(
                                hT[:], h[:], identf[:], is_transpose=True,
                                start=True, stop=True, skip_group_check=True,
                            )
                        else:
                            # PE-fused: 16 accumulating bf16 transpose passes
                            # (CoreSim-only -- bf16 PSUM accumulation is not
                            # trustworthy on real HW)
                            hT = p1t.tile([128, 128], bf16, tag="hT")
                            for n in range(NCMP):
                                nc.tensor.matmul(
                                    hT[:], tv[:, :, n], ident16[:],
                                    is_transpose=True,
                                    start=(n == 0), stop=(n == NCMP - 1),
                                    skip_group_check=True,
                                )
                        hTs = wk4.tile([128, 128], bf16, tag="hTs")
                        nc.scalar.copy(hTs[:], hT[:])
                        nc.sync.dma_start(
                            cc_in[:, r * SHARD + t * 128 : r * SHARD + (t + 1) * 128],
                            hTs[:],
                        )

                for t in range(NT4):
                    w3ps = p1w.tile([128, 48], f32, tag="w3ps")
                    for dk in range(8):
                        nc.tensor.matmul(
                            w3ps[:], xt[:, dk, ts(t, 128)], rt[:, dk, :],
                            start=(dk == 0), stop=(dk == 7),
                        )
                    for r in range(3):
                        # no max-subtraction: router logits are O(1)
                        e3 = wk4.tile([128, 16], f32, tag="e3")
                        z3 = wk4.tile([128, 1], f32, tag="z3")
                        nc.scalar.activation(
                            e3[:], w3ps[:, ts(r, 16)], Exp,
                            bias=0.0, scale=1.0, accum_out=z3[:],
                        )
                        rz3 = wk4.tile([128, 1], f32, tag="rz3")
                        nc.vector.reciprocal(rz3[:], z3[:])
                        nc.vector.tensor_scalar_mul(
                            w3nb[:, t, ts(r, 16)], e3[:], rz3[:]
                        )

                    # y[s, (r, n)] = x @ C  (r outer, n inner), 2 chunks/bank-pair
                    for cp in range(2):
                        yp = p1y.tile([128, 2, 512], f32, tag="yp")
                        for c2 in range(2):
                            ch = cp * 2 + c2
                            for dk in range(8):
                                nc.tensor.matmul(
                                    yp[:, c2, :], xt[:, dk, ts(t, 128)],
                                    cfl[:, dk, ts(ch, 512)],
                                    start=(dk == 0), stop=(dk == 7),
                                )
                        nc.scalar.copy(ys[t][:, ts(cp, 1024)], yp[:])
                    if t > 0:
                        emit_combine(t - 1)
                emit_combine(NT4 - 1)
                # single merged AllGather for all 3 routers
                nc.gpsimd.collective_compute(
                    "AllGather", mybir.AluOpType.bypass,
                    replica_groups=groups,
                    ins=[cc_in[:]], outs=[cc_out[:]],
                )

            if KNOB_STOP == "p1":
                # timing bisect: consume cc_out, skip expand+attention
                dumb = sbp.tile([128, 512], bf16)
                nc.sync.dma_start(
                    dumb[:],
                    bass.AP(
                        tensor=cc_out.tensor, offset=cc_out.offset,
                        ap=[[3 * SHARD, 128], [1, 512]],
                    ),
                )
                nc.sync.dma_start(out_d[0:128, 0:512], dumb[:])
                continue

            # ------------- expand: Q^T, K^T, V' -------------
            hxt = sbp.tile([128, 3, S], bf16)
            for r in range(3):
                srcap = bass.AP(
                    tensor=cc_out.tensor,
                    offset=cc_out.offset + r * SHARD,
                    ap=[[3 * SHARD, 128], [128 * 3 * SHARD, 4], [1, SHARD]],
                )
                nc.sync.dma_start(hxt[:, r, :], srcap)

            QT = sbp.tile([128, 2, S], bf16)
            KT = sbp.tile([128, 2, S], bf16)
            with tc.tile_pool(name="qkps", bufs=3, space="PSUM") as qkps:
                for r, dst in ((0, QT), (1, KT)):
                    for pair in range(2):
                        for ch in range(4):
                            ps = qkps.tile([128, 512], f32, tag="qk")
                            nc.tensor.matmul(
                                ps[:], wq[:, r, ts(pair, 128)],
                                hxt[:, r, ts(ch, 512)],
                                start=True, stop=True,
                            )
                            nc.vector.tensor_copy(dst[:, pair, ts(ch, 512)], ps[:])
                for st in range(KT_TILES):
                    vps = qkps.tile([128, HPC * DH], f32, tag="qk")
                    nc.tensor.matmul(
                        vps[:], hxt[:, 2, ts(st, 128)], wq[:, 2, :],
                        start=True, stop=True,
                    )
                    nc.vector.tensor_copy(
                        vV[:, st, :, 0:64],
                        vps[:].rearrange("p (h d) -> p h d", d=DH),
                    )

            # ------------- attention + wO -------------
            with (
                tc.tile_pool(name="stp", bufs=2, space="PSUM") as stp,
                tc.tile_pool(name="avp", bufs=2, space="PSUM") as avp,
                tc.tile_pool(name="bcp", bufs=1, space="PSUM") as bcp,
                tc.tile_pool(name="opp", bufs=1, space="PSUM") as opp,
                tc.tile_pool(name="wkp", bufs=3) as wkp,
            ):

                def emit_norm(qi, av):
                    """Normalize a finished q-tile into a pair-stacked aoT.
                    Deferred into the next q-tile's issue window so the queues
                    never stall on the AV->recip->bc->mul chain."""
                    rzt = wk2.tile([128, HPC, 128], bf16, tag="rzt")
                    # 1/z for all 4 heads in one op (z rows on partition 64).
                    # The extra copy matters: a non-DVE engine waiting directly
                    # on a Reciprocal's completion sem stalls ~600us on HW
                    # (event-accel pathology); the PE waits on the copy instead.
                    if KNOB_TAILX in ("norec", "nonorm"):
                        nc.vector.memset(rzt[64:65, :, :], 1.0)
                    else:
                        # 1/z = exp(-ln z) on the Act engine (Ln and Exp share
                        # an activation-table set, so no table swaps).  This
                        # avoids the DVE Reciprocal instruction entirely --
                        # chains consuming its output stall for hundreds of
                        # microseconds per occurrence on real HW.
                        rln = wk4.tile([128, HPC, 128], f32, tag="rln")
                        nc.scalar.activation(
                            rln[64:65, :, :], av[64:65, :, :], Ln,
                            bias=0.0, scale=1.0,
                        )
                        nc.scalar.activation(
                            rzt[64:65, :, :], rln[64:65, :, :], Exp,
                            bias=0.0, scale=-1.0,
                        )
                    # broadcast 1/z to 64 rows (PE outer product, or gpsimd
                    # partition_broadcast), then normalize each head's [64, q]
                    # tile into bf16.  The PSUM bank also hosts the odd-head
                    # partition-shift staging on partitions 64..127.
                    bsh = bcp.tile([128, HPC, 128], f32, tag="bc")
                    bcs = wk2.tile([64, HPC, 128], bf16, tag="bcs")
                    if KNOB_TAILX in ("nobc", "nonorm"):
                        nc.vector.memset(bcs[:], 1.0)
                    elif KNOB_PB:
                        nc.gpsimd.partition_broadcast(
                            bcs[:], rzt[64:65, :, :], channels=64
                        )
                    else:
                        for h in range(HPC):
                            nc.tensor.matmul(
                                bsh[0:64, h, :], ones64[64:65, :],
                                rzt[64:65, h, :],
                                start=True, stop=True, skip_group_check=True,
                            )
                        nc.vector.tensor_copy(bcs[:], bsh[0:64, :, :])
                    # pair-stacked aoT for a K=128 wO: even heads' normalized
                    # tiles are written in place (partitions 0..63); odd heads
                    # go to a scratch tile and a PE identity-matmul moves both
                    # to partitions 64..127 (compute engines are lane-locked,
                    # but the PE can write any PSUM partition range)
                    if KNOB_SHIFT:
                        aoT = wk2.tile([128, 2, 128], bf16, tag="aoT")
                        if KNOB_SHIFT == 2:
                            aodd = aoddp[:, qi % 2, :, :]
                        else:
                            aodd_t = wk2.tile([64, 2, 128], bf16, tag="aodd")
                            aodd = aodd_t[:]
                        for pair in range(2):
                            nc.vector.tensor_mul(
                                aodd[0:64, pair, :], av[0:64, 2 * pair + 1, :],
                                bcs[:, 2 * pair + 1, :],
                            )
                        for pair in range(2):
                            if KNOB_SHIFT == 2:
                                nc.tensor.matmul(
                                    bsh[64:128, pair, :], ident16[:, 0:64],
                                    aodd[:, pair, :],
                                    start=True, stop=True,
                                    skip_group_check=True,
                                )
                            else:
                                nc.tensor.matmul(
                                    bsh[64:128, pair, :], ident16[0:64, 0:64],
                                    aodd[0:64, pair, :],
                                    start=True, stop=True,
                                    skip_group_check=True,
                                )
                        nc.scalar.copy(aoT[64:128, :, :], bsh[64:128, 0:2, :])
                        for pair in range(2):
                            nc.vector.tensor_mul(
                                aoT[0:64, pair, :], av[0:64, 2 * pair, :],
                                bcs[:, 2 * pair, :],
                            )
                    else:
                        aoT = wk2.tile([64, HPC, 128], bf16, tag="aoT")
                        for h in range(HPC):
                            nc.vector.tensor_mul(
                                aoT[:, h, :], av[0:64, h, :], bcs[:, h, :]
                            )
                    return aoT

                def emit_wo(qi, aoT):
                    for e in range(2):
                        ops = opp.tile([128, 512], f32, tag="o")
                        if KNOB_SHIFT:
                            for pair in range(2):
                                nc.tensor.matmul(
                                    ops[:], aoT[:, pair, :],
                                    wo[:, pair, ts(e, 512)],
                                    start=(pair == 0), stop=(pair == 1),
                                    skip_group_check=True,
                                )
                        else:
                            for h in range(HPC):
                                nc.tensor.matmul(
                                    ops[:], aoT[:, h, :], wo4[:, h, ts(e, 512)],
                                    start=(h == 0), stop=(h == 3),
                                    skip_group_check=True,
                                )
                        osb = wk2.tile([128, 512], bf16, tag="osb")
                        nc.vector.tensor_copy(osb[:], ops[:])
                        nc.sync.dma_start(out_d[ts(qi, 128), ts(e, 512)], osb[:])

                def emit_scores(qi, h, nkt, diag_kt, customs):
                    """QK^T + exp for one (q-tile, head); returns the P tile."""
                    pair, doff = h // 2, (h % 2) * 64
                    p_sb = wkp.tile([128, KT_TILES, 128], bf16, tag="p")
                    for kb in range(0, nkt, 8):
                        kw = min(8, nkt - kb)
                        st_ps = stp.tile([128, 8, 128], f32, tag="st")
                        for j in range(kw):
                            nc.tensor.matmul(
                                st_ps[:, j, :],
                                KT[doff : doff + 64, pair, ts(kb + j, 128)],
                                QT[doff : doff + 64, pair, ts(qi, 128)],
                                start=True, stop=True,
                                skip_group_check=True,
                            )
                        for kt, mi in customs:
                            if kb <= kt < kb + kw:
                                nc.vector.tensor_add(
                                    st_ps[:, kt - kb, :],
                                    st_ps[:, kt - kb, :],
                                    masks[:, mi, :],
                                )
                        # exp(score - 20); the shift cancels in the
                        # softmax normalization
                        nc.scalar.activation(
                            p_sb[:, kb : kb + kw, :], st_ps[:, 0:kw, :],
                            Exp, bias=cbias[:], scale=1.0,
                        )
                        if diag_kt is not None and kb <= diag_kt < kb + kw:
                            # zero P where k > q
                            if KNOB_AFF:
                                nc.gpsimd.affine_select(
                                    p_sb[:, diag_kt, :], p_sb[:, diag_kt, :],
                                    pattern=[[1, 128]],
                                    compare_op=mybir.AluOpType.is_ge,
                                    fill=0.0,
                                    base=0,
                                    channel_multiplier=-1,
                                )
                            else:
                                # gpsimd standard-library TensorTensor: the
                                # Pool queue is idle during attention
                                nc.gpsimd.tensor_mul(
                                    p_sb[:, diag_kt, :], p_sb[:, diag_kt, :],
                                    triu[:],
                                )
                    return p_sb

                def emit_av(av, h, nkt, p_sb):
                    for kt in range(nkt):
                        nc.tensor.matmul(
                            av[:, h, :],
                            vV[:, kt, h, :],
                            p_sb[:, kt, :],
                            start=(kt == 0), stop=(kt == nkt - 1),
                            skip_group_check=True,
                        )

                # slot-based software pipeline over the 64 (q-tile, head)
                # iterations: AV lags its scores by 2 slots (hides the Act exp
                # latency), each tile's normalize chain runs one slot after its
                # last AV, and its wO two slots later -- so no engine queue
                # ever sits on a cross-engine dependency.
                slot = 0
                due = []  # (due_slot, fn), FIFO per slot

                def sched(delay, fn):
                    due.append((slot + delay, fn))

                def run_due():
                    i = 0
                    while i < len(due):
                        s, fn = due[i]
                        if s <= slot:
                            due.pop(i)
                            fn()
                        else:
                            i += 1

                def make_av(av, h, nkt, p_sb):
                    return lambda: emit_av(av, h, nkt, p_sb)

                def make_norm(qi, av, box):
                    def fn():
                        box.append(emit_norm(qi, av))

                    return fn

                def make_wo(qi, box):
                    return lambda: emit_wo(qi, box[0])

                # alternate big/small tiles to smooth the engine mix
                _ord = []
                _big = sorted(range(QT_TILES), key=lambda q: -plan[q][0])
                for _i in range(QT_TILES // 2):
                    _ord += [_big[_i], _big[QT_TILES - 1 - _i]]
                for qi in _ord:
                    nkt, diag_kt, customs = plan[qi]
                    av = avp.tile([65, HPC, 128], f32, tag="av")
                    for h in range(HPC):
                        p_sb = emit_scores(qi, h, nkt, diag_kt, customs)
                        if KNOB_STOP != "qk":
                            sched(2, make_av(av, h, nkt, p_sb))
                        if h == 3 and not KNOB_STOP:
                            box = []
                            sched(3, make_norm(qi, av, box))
                            sched(5, make_wo(qi, box))
                        slot += 1
                        run_due()
                # flush
                slot += 16
                run_due()
                if KNOB_STOP in ("qk", "av"):
                    dumb2 = sbp.tile([128, 512], bf16)
                    nc.vector.tensor_copy(dumb2[:], QT[:, 0, 0:512])
                    nc.sync.dma_start(out_d[0:128, 0:512], dumb2[:])

    return nc


def _make_runner(plan, nt, repeat=1):
    """Compile the graph once and return fn(in_maps) -> list of out arrays."""
    import jax
    import numpy as np
    from jax.sharding import Mesh, PartitionSpec
    from jax.experimental.shard_map import shard_map
    import concourse.bass2jax as bass2jax
    import concourse.mybir as mybir

    nc = _build(plan, nt, repeat=repeat)
    bass2jax.install_neuronx_cc_hook()

    partition_name = nc.partition_id_tensor.name if nc.partition_id_tensor else None
    in_names, out_names, out_avals = [], [], []
    for alloc in nc.m.functions[0].allocations:
        if not isinstance(alloc, mybir.MemoryLocationSet):
            continue
        name = alloc.memorylocations[0].name
        if alloc.kind == "ExternalInput":
            if name != partition_name:
                in_names.append(name)
        elif alloc.kind == "ExternalOutput":
            out_names.append(name)
            out_avals.append(
                jax.core.ShapedArray(
                    tuple(alloc.tensor_shape), mybir.dt.np(alloc.dtype)
                )
            )
    all_names = in_names + out_names
    if partition_name is not None:
        all_names = all_names + [partition_name]

    def _body(*args):
        operands = list(args)
        if partition_name is not None:
            operands.append(bass2jax.partition_id_tensor())
        outs = bass2jax._bass_exec_p.bind(
            *operands,
            out_avals=tuple(out_avals),
            in_names=tuple(all_names),
            out_names=tuple(out_names),
            lowering_input_output_aliases=(),
            sim_require_finite=True,
            sim_require_nnan=True,
            nc=nc,
        )
        return tuple(outs)

    devices = jax.devices()[:N_CORES]
    mesh = Mesh(np.asarray(devices), ("core",))
    SHARED = {"cflat", "routersT", "dmask"}
    in_specs = tuple(
        PartitionSpec() if n in SHARED else PartitionSpec("core") for n in in_names
    ) + (PartitionSpec("core"),) * len(out_names)
    sharded = jax.jit(
        shard_map(
            _body,
            mesh=mesh,
            in_specs=in_specs,
            out_specs=(PartitionSpec("core"),) * len(out_names),
            check_rep=False,
        ),
        keep_unused=True,
    )
    zeros = [
        np.zeros((N_CORES * a.shape[0], *a.shape[1:]), a.dtype) for a in out_avals
    ]

    def make_args(in_maps, device=False):
        arrs = []
        for n in in_names:
            if n in SHARED:
                arrs.append(np.asarray(in_maps[0][n]))
            else:
                arrs.append(
                    np.concatenate([np.asarray(m[n]) for m in in_maps], axis=0)
                )
        arrs += list(zeros)
        if device:
            from jax.sharding import NamedSharding

            for i, n in enumerate(in_names):
                sh = NamedSharding(
                    mesh, PartitionSpec() if n in SHARED else PartitionSpec("core")
                )
                arrs[i] = jax.device_put(arrs[i], sh)
            sh = NamedSharding(mesh, PartitionSpec("core"))
            for i in range(len(in_names), len(arrs)):
                arrs[i] = jax.device_put(arrs[i], sh)
        return arrs

    def run(in_maps):
        outs = sharded(*make_args(in_maps))
        res = np.asarray(outs[out_names.index("out")])
        return res.reshape(N_CORES, S, D)

    run.sharded = sharded
    run.make_args = make_args
    run.out_index = out_names.index("out")
    return run


def _prepare(inputs):
    """Host-side prep: mask plan + per-core input maps."""
    x = np.asarray(inputs["x"], np.float32)
    mask = np.asarray(inputs["mask"], bool)[0, 0]
    compress = np.asarray(inputs["compress_neurons"], np.float32)
    rQ = np.asarray(inputs["router_Q"], np.float32)
    rK = np.asarray(inputs["router_K"], np.float32)
    rV = np.asarray(inputs["router_V"], np.float32)
    wQ = np.asarray(inputs["wQ"], np.float32)
    wK = np.asarray(inputs["wK"], np.float32)
    wV = np.asarray(inputs["wV"], np.float32)
    wO = np.asarray(inputs["wO"], np.float32)

    plan, mtiles = _mask_plan(mask)
    nt = len(mtiles)

    # host-side shared prep
    import ml_dtypes

    bf = ml_dtypes.bfloat16
    cflat = np.ascontiguousarray(
        compress.transpose(1, 2, 0).reshape(8, 128, RANK * NCMP)
    ).astype(bf)  # [D, R, NC] -> d-tiles
    routersT = np.ascontiguousarray(
        np.stack([rQ, rK, rV]).transpose(2, 0, 1).reshape(8, 128, 48)
    ).astype(bf)
    wqT = wQ.T * np.float32(1.0 / np.sqrt(DH))  # fold 1/sqrt(dh) into Q
    wkT, wvT = wK.T, wV.T
    wOT = np.ascontiguousarray(wO.T).astype(bf)  # [D, E]

    in_maps = []
    for c in range(N_CORES):
        b, q4 = divmod(c, 4)
        hs = slice(HPC * q4 * DH, HPC * q4 * DH + HPC * DH)
        m = {
            "xT": np.ascontiguousarray(x[b, q4 * SHARD : (q4 + 1) * SHARD, :].T)
            .reshape(8, 128, SHARD)
            .astype(bf),
            "cflat": cflat,
            "routersT": routersT,
            "wqkvT": np.ascontiguousarray(
                np.stack([wqT[:, hs], wkT[:, hs], wvT[:, hs]])
            ).astype(bf),
            "wOT": np.ascontiguousarray(wOT[hs, :]).reshape(2, 128, D),
        }
        if nt:
            m["dmask"] = mtiles
        in_maps.append(m)
    return plan, nt, in_maps


def kernel(**inputs):
    plan, nt, in_maps = _prepare(inputs)
    key = (plan, nt)
    if key not in _RUNNERS:
        _RUNNERS[key] = _make_runner(plan, nt)
    res = _RUNNERS[key](in_maps)  # [8, S, D] bf16 partials
    out = np.empty((B, S, D), np.float32)
    for b in range(B):
        out[b] = res[4 * b : 4 * b + 4].astype(np.float32).sum(axis=0)
    return out


# revision 67
# speedup vs baseline: 34.4309x; 1.0482x over previous
"""MoE-routed low-rank attention (nn_NeuronCircuit_28930899706061) on 8 TRN2 cores.

Sharding: core c in 0..7 -> batch b = c//4, token-shard q4 = c%4 (512 tokens)
for the compress/routing phase; head group heads [4*q4, 4*q4+4) of batch b for
the attention phase. h^T tensors for all 3 routers are all-gathered in ONE
collective within each 4-core batch group; each core emits a partial [S, D]
output (its 4 heads' contribution through wO) and the host sums the 4 partials
per batch.

Attention computes scores TRANSPOSED ([k, q] with k on partitions) so the
softmax-weighted AV needs no P-transpose: AV contracts k on partitions
directly, and the softmax denominator comes for free from a ones-column
appended to V (row 64/63 of the AV PSUM accumulator). Normalization is a
PE outer-product broadcast of 1/z plus one DVE multiply per head.

All weight transposes are done host-side so every device DMA is contiguous.
"""

import os

import numpy as np

# ablation knobs (default = fast path); set to "0" to fall back
# PE transpose-accum combine: correct in CoreSim but WRONG on real HW (bf16
# matmul outputs cannot accumulate in PSUM banks there) -- keep off
KNOB_TA = os.environ.get("KNOB_TA", "0") == "1"
# 0: K=64 per-head wO (no partition move, validated); 1: K=64-lhsT PE shift
# (broken on HW); 2: K=128 baseline-shape PE shift
KNOB_SHIFT = int(os.environ.get("KNOB_SHIFT", "1"))
# affine_select with channel_multiplier=-1: accepted by CoreSim but silently
# wrong on real HW ucode -- default to a multiplicative triangle mask instead
KNOB_AFF = os.environ.get("KNOB_AFF", "0") == "1"
KNOB_PB = os.environ.get("KNOB_PB", "0") == "1"  # gpsimd partition_broadcast
KNOB_STOP = os.environ.get("KNOB_STOP", "")  # "p1": stop after gather
KNOB_TAILX = os.environ.get("KNOB_TAILX", "")  # norec|nobc|nonorm ablations

B, S, D, H, RANK, NCMP = 2, 2048, 1024, 16, 128, 16
DH = D // H  # 64
N_CORES = 8
SHARD = S // 4  # 512 tokens per core in phase 1
HPC = 4  # heads per core
QT_TILES = S // 128  # 16 q tiles
KT_TILES = S // 128  # 16 k tiles

_RUNNERS: dict = {}


def _split_multi_waits(nc, mybir):
    """This toolchain's walrus rejects any instruction carrying >1 sync wait
    ("Too many sync wait commands"); hoist excess waits onto same-engine nops
    inserted immediately before the instruction."""
    cnt = 0
    for f in nc.m.functions:
        for blk in f.blocks:
            il = blk.instructions
            out = []
            changed = False
            for inst in il:
                si = inst.sync_info
                waits = list(si.on_wait or []) if si else []
                if len(waits) > 1:
                    for w in waits[:-1]:
                        cnt += 1
                        nop = mybir.InstNoOp(
                            name=f"wsplit-{cnt}",
                            engine=inst.engine,
                            sync_info=mybir.SyncInfo(on_wait=[w], on_update=[]),
                        )
                        nc.register_instruction(nop)
                        out.append(nop)
                    inst.sync_info = mybir.SyncInfo(
                        on_wait=[waits[-1]], on_update=list(si.on_update or [])
                    )
                    changed = True
                out.append(inst)
            if changed:
                il[:] = out


def _make_tc_class(tile, mybir):
    class TC(tile.TileContext):
        def __exit__(self, *exc):
            ret = super().__exit__(*exc)
            if exc[0] is None:
                _split_multi_waits(self.nc, mybir)
            return ret

    return TC


_TRI = np.tril(np.ones((128, 128), dtype=bool))  # keep k<=q in [k,q] layout is triu


def _mask_plan(maskb):
    """Per q-tile: (nkt, diag_kt, customs) in TRANSPOSED [k, q] tile terms.

    nkt: number of live 128-wide k tiles; diag_kt: k-tile index that is
    exactly the causal triangle (k<=q kept), or None; customs: list of
    (kt, mask_tile_idx) for other partial tiles.  Returns (plan, tiles)
    with tiles a [nt, 128, 128] f32 array of ADDITIVE masks in [k, q]
    orientation.
    """
    tiles = []
    tile_ids = {}
    plan = []
    for qi in range(QT_TILES):
        rows = maskb[qi * 128 : (qi + 1) * 128]  # [128 q, S k]
        nkt = 0
        for kt in range(KT_TILES):
            if rows[:, kt * 128 : (kt + 1) * 128].any():
                nkt = kt + 1
        diag_kt = None
        customs = []
        for kt in range(nkt):
            sub = rows[:, kt * 128 : (kt + 1) * 128]  # [q, k]
            if sub.all():
                continue
            subT = sub.T  # [k, q]
            if diag_kt is None and (subT == _TRI.T).all():
                # exactly "keep k <= q within tile"
                diag_kt = kt
                continue
            add = np.where(subT, np.float32(0), np.float32(-1e30))
            key = add.tobytes()
            if key not in tile_ids:
                tile_ids[key] = len(tiles)
                tiles.append(add)
            customs.append((kt, tile_ids[key]))
        plan.append((nkt, diag_kt, tuple(customs)))
    nt = len(tiles)
    tiles_arr = (
        np.stack(tiles).astype(np.float32)
        if nt
        else np.zeros((0, 128, 128), np.float32)
    )
    return tuple(plan), tiles_arr


def _bcast_mid(bass, ap, n):
    """[P, K] AP -> [P, n, K] AP with a step-0 middle dim (free broadcast)."""
    dims = [list(x) for x in ap.ap]
    return bass.AP(
        tensor=ap.tensor, offset=ap.offset, ap=[dims[0], [0, n]] + dims[1:]
    )


def _build(plan, nt, repeat=1):
    import concourse.bass as bass
    import concourse.mybir as mybir
    import concourse.tile as tile
    from concourse.bass import ts
    from concourse.masks import make_identity, make_upper_triangular

    f32 = mybir.dt.float32
    bf16 = mybir.dt.bfloat16
    Exp = mybir.ActivationFunctionType.Exp
    Ln = mybir.ActivationFunctionType.Ln
    TC = _make_tc_class(tile, mybir)

    nc = bass.Bass(num_devices=N_CORES)
    xT_d = nc.dram_tensor("xT", [8, 128, SHARD], bf16, kind="ExternalInput")
    cflat_d = nc.dram_tensor("cflat", [8, 128, RANK * NCMP], bf16, kind="ExternalInput")
    routersT_d = nc.dram_tensor("routersT", [8, 128, 48], bf16, kind="ExternalInput")
    wqkvT_d = nc.dram_tensor("wqkvT", [3, 128, HPC * DH], bf16, kind="ExternalInput")
    wOT_d = nc.dram_tensor("wOT", [2, 128, D], bf16, kind="ExternalInput")
    dmask_d = (
        nc.dram_tensor("dmask", [nt, 128, 128], f32, kind="ExternalInput")
        if nt
        else None
    )
    out_d = nc.dram_tensor("out", [S, D], bf16, kind="ExternalOutput")

    groups = [[0, 1, 2, 3], [4, 5, 6, 7]]
    NT4 = SHARD // 128  # 4 s-tiles per core in phase 1

    with TC(nc) as tc:
      for _rep in range(repeat):
        with (
            tc.tile_pool(name="sb", bufs=1) as sbp,
            tc.tile_pool(name="wk2", bufs=2) as wk2,
            tc.tile_pool(name="wk4", bufs=4) as wk4,
            tc.tile_pool(name="dram", bufs=1, space="DRAM") as dramp,
        ):
            ident16 = sbp.tile([128, 128], bf16)
            make_identity(nc, ident16)
            # multiplicative causal mask for [k, q] diagonal tiles:
            # 1.0 where k <= q, 0.0 where k > q
            triu = sbp.tile([128, 128], bf16)
            make_upper_triangular(nc, triu[:], val=1.0, diag=True)
            if not KNOB_TA:
                identf = sbp.tile([128, 128], f32)
                make_identity(nc, identf)
            ones64 = sbp.tile([128, 64], bf16)
            nc.vector.memset(ones64[:], 1.0)
            cbias = sbp.tile([128, 1], f32)
            nc.vector.memset(cbias[:], -20.0)
            cc_in = dramp.tile([128, 3 * SHARD], bf16, name="cc_in")
            cc_out = dramp.tile([4, 128, 3 * SHARD], bf16, name="cc_out")

            # phase-1 inputs first on the DMA queue (they gate the pipeline)
            xt = sbp.tile([128, 8, SHARD], bf16)
            rt = sbp.tile([128, 8, 48], bf16)
            cfl = sbp.tile([128, 8, RANK * NCMP], bf16)
            nc.sync.dma_start(xt[:], xT_d[:].rearrange("d p s -> p d s"))
            nc.sync.dma_start(rt[:], routersT_d[:].rearrange("d p s -> p d s"))
            nc.sync.dma_start(
                cfl[:, :, ts(0, 512)],
                cflat_d[:].rearrange("d p s -> p d s")[:, :, ts(0, 512)],
            )
            # attention-side constant loads
            wq = sbp.tile([128, 3, HPC * DH], bf16)
            for r in range(3):
                nc.sync.dma_start(wq[:, r, :], wqkvT_d[r])
            if KNOB_SHIFT:
                wo = sbp.tile([128, 2, D], bf16)
                for k in range(2):
                    nc.sync.dma_start(wo[:, k, :], wOT_d[k])
            else:
                # per-head wO rows on partitions 0..63 (K=64 accumulation)
                wo4 = sbp.tile([64, HPC, D], bf16)
                for k in range(2):
                    nc.sync.dma_start(
                        wo4[:, 2 * k : 2 * k + 2, :],
                        wOT_d[k].rearrange("(h p) e -> p h e", h=2),
                    )
            # remaining compress-weight chunks
            for ch in range(1, 4):
                for dk in range(8):
                    nc.sync.dma_start(
                        cfl[:, dk, ts(ch, 512)], cflat_d[dk][:, ts(ch, 512)]
                    )
            if nt:
                masks = sbp.tile([128, nt, 128], f32)
                for t in range(nt):
                    nc.sync.dma_start(masks[:, t, :], dmask_d[t])
            # V' tile: per (k-tile, head): [V | 1]; the ones column makes
            # row 64 of the AV accumulator the softmax denominator z
            vV = sbp.tile([128, KT_TILES, HPC, 65], bf16)
            nc.gpsimd.memset(vV[:, :, :, 64:65], 1.0)
            if KNOB_SHIFT == 2:
                # persistent double-buffered odd-head scratch; rows 64..127
                # stay zero so the K=128 ident-shift reads clean zeros there
                aoddp = sbp.tile([128, 2, 2, 128], bf16)
                nc.vector.memset(aoddp[:], 0.0)
            if KNOB_PB:
                from concourse.library_config import attn as _attnlib

                nc.gpsimd.load_library(_attnlib)

            with (
                tc.tile_pool(name="p1y", bufs=2, space="PSUM") as p1y,
                tc.tile_pool(name="p1w", bufs=1, space="PSUM") as p1w,
                tc.tile_pool(name="p1t", bufs=2, space="PSUM") as p1t,
            ):
                # ------------- phase 1: routing + compress -------------

                ys = [
                    sbp.tile([128, RANK * NCMP], bf16, name=f"y{t}")
                    for t in range(NT4)
                ]
                w3nb = sbp.tile([128, NT4, 48], bf16)

                def emit_combine(t):
                    """Weighted neuron-sum + transpose + gather-staging for one
                    finished s-tile.  Deferred one tile so the PE never stalls
                    on the DVE multiply."""
                    for r in range(3):
                        yv = ys[t][:].rearrange("p (r n) -> p r n", n=NCMP)
                        tmp = wk2.tile([128, RANK * NCMP], bf16, tag="tmp")
                        tv = tmp[:].rearrange("p (r n) -> p r n", n=NCMP)
                        wb = _bcast_mid(bass, w3nb[:, t, ts(r, 16)], RANK)
                        if r >= 1:
                            # gpsimd TensorTensor (standard library) relieves
                            # the DVE, which carries the grouped reduces
                            nc.gpsimd.tensor_mul(tv, yv, wb)
                        else:
                            nc.vector.tensor_mul(tv, yv, wb)
                        if not KNOB_TA:
                            # HW-validated: DVE grouped reduce + f32 transpose
                            hT = p1t.tile([128, 128], f32, tag="hT")
                            h = wk4.tile([128, RANK], f32, tag="h")
                            nc.vector.tensor_reduce(
                                h[:], tv, axis=mybir.AxisListType.X,
                                op=mybir.AluOpType.add,
                            )
                            nc.tensor.matmul(
                                hT[:], h[:], identf[:], is_transpose=True,
                                start=True, stop=True, skip_group_check=True,
                            )
                        else:
                            # PE-fused: 16 accumulating bf16 transpose passes
                            # (CoreSim-only -- bf16 PSUM accumulation is not
                            # trustworthy on real HW)
                            hT = p1t.tile([128, 128], bf16, tag="hT")
                            for n in range(NCMP):
                                nc.tensor.matmul(
                                    hT[:], tv[:, :, n], ident16[:],
                                    is_transpose=True,
                                    start=(n == 0), stop=(n == NCMP - 1),
                                    skip_group_check=True,
                                )
                        hTs = wk4.tile([128, 128], bf16, tag="hTs")
                        nc.scalar.copy(hTs[:], hT[:])
                        nc.sync.dma_start(
                            cc_in[:, r * SHARD + t * 128 : r * SHARD + (t + 1) * 128],
                            hTs[:],
                        )

                for t in range(NT4):
                    w3ps = p1w.tile([128, 48], f32, tag="w3ps")
                    for dk in range(8):
                        nc.tensor.matmul(
                            w3ps[:], xt[:, dk, ts(t, 128)], rt[:, dk, :],
                            start=(dk == 0), stop=(dk == 7),
                        )
                    for r in range(3):
                        # no max-subtraction: router logits are O(1)
                        e3 = wk4.tile([128, 16], f32, tag="e3")
                        z3 = wk4.tile([128, 1], f32, tag="z3")
                        nc.scalar.activation(
                            e3[:], w3ps[:, ts(r, 16)], Exp,
                            bias=0.0, scale=1.0, accum_out=z3[:],
                        )
                        rz3 = wk4.tile([128, 1], f32, tag="rz3")
                        nc.vector.reciprocal(rz3[:], z3[:])
                        nc.vector.tensor_scalar_mul(
                            w3nb[:, t, ts(r, 16)], e3[:], rz3[:]
                        )

                    # y[s, (r, n)] = x @ C  (r outer, n inner), 2 chunks/bank-pair
                    for cp in range(2):
                        yp = p1y.tile([128, 2, 512], f32, tag="yp")
                        for c2 in range(2):
                            ch = cp * 2 + c2
                            for dk in range(8):
                                nc.tensor.matmul(
                                    yp[:, c2, :], xt[:, dk, ts(t, 128)],
                                    cfl[:, dk, ts(ch, 512)],
                                    start=(dk == 0), stop=(dk == 7),
                                )
                        nc.scalar.copy(ys[t][:, ts(cp, 1024)], yp[:])
                    if t > 0:
                        emit_combine(t - 1)
                emit_combine(NT4 - 1)
                # single merged AllGather for all 3 routers
                nc.gpsimd.collective_compute(
                    "AllGather", mybir.AluOpType.bypass,
                    replica_groups=groups,
                    ins=[cc_in[:]], outs=[cc_out[:]],
                )

            if KNOB_STOP == "p1":
                # timing bisect: consume cc_out, skip expand+attention
                dumb = sbp.tile([128, 512], bf16)
                nc.sync.dma_start(
                    dumb[:],
                    bass.AP(
                        tensor=cc_out.tensor, offset=cc_out.offset,
                        ap=[[3 * SHARD, 128], [1, 512]],
                    ),
                )
                nc.sync.dma_start(out_d[0:128, 0:512], dumb[:])
                continue

            # ------------- expand: Q^T, K^T, V' -------------
            hxt = sbp.tile([128, 3, S], bf16)
            for r in range(3):
                srcap = bass.AP(
                    tensor=cc_out.tensor,
                    offset=cc_out.offset + r * SHARD,
                    ap=[[3 * SHARD, 128], [128 * 3 * SHARD, 4], [1, SHARD]],
                )
                nc.sync.dma_start(hxt[:, r, :], srcap)

            QT = sbp.tile([128, 2, S], bf16)
            KT = sbp.tile([128, 2, S], bf16)
            with tc.tile_pool(name="qkps", bufs=3, space="PSUM") as qkps:
                for r, dst in ((0, QT), (1, KT)):
                    for pair in range(2):
                        for ch in range(4):
                            ps = qkps.tile([128, 512], f32, tag="qk")
                            nc.tensor.matmul(
                                ps[:], wq[:, r, ts(pair, 128)],
                                hxt[:, r, ts(ch, 512)],
                                start=True, stop=True,
                            )
                            nc.vector.tensor_copy(dst[:, pair, ts(ch, 512)], ps[:])
                for st in range(KT_TILES):
                    vps = qkps.tile([128, HPC * DH], f32, tag="qk")
                    nc.tensor.matmul(
                        vps[:], hxt[:, 2, ts(st, 128)], wq[:, 2, :],
                        start=True, stop=True,
                    )
                    nc.vector.tensor_copy(
                        vV[:, st, :, 0:64],
                        vps[:].rearrange("p (h d) -> p h d", d=DH),
                    )

            # ------------- attention + wO -------------
            with (
                tc.tile_pool(name="stp", bufs=2, space="PSUM") as stp,
                tc.tile_pool(name="avp", bufs=2, space="PSUM") as avp,
                tc.tile_pool(name="bcp", bufs=1, space="PSUM") as bcp,
                tc.tile_pool(name="opp", bufs=1, space="PSUM") as opp,
                tc.tile_pool(name="wkp", bufs=3) as wkp,
            ):

                def emit_norm(qi, av):
                    """Normalize a finished q-tile into a pair-stacked aoT.
                    Deferred into the next q-tile's issue window so the queues
                    never stall on the AV->recip->bc->mul chain."""
                    rzt = wk2.tile([128, HPC, 128], bf16, tag="rzt")
                    # 1/z for all 4 heads in one op (z rows on partition 64).
                    # The extra copy matters: a non-DVE engine waiting directly
                    # on a Reciprocal's completion sem stalls ~600us on HW
                    # (event-accel pathology); the PE waits on the copy instead.
                    if KNOB_TAILX in ("norec", "nonorm"):
                        nc.vector.memset(rzt[64:65, :, :], 1.0)
                    else:
                        # 1/z = exp(-ln z) on the Act engine (Ln and Exp share
                        # an activation-table set, so no table swaps).  This
                        # avoids the DVE Reciprocal instruction entirely --
                        # chains consuming its output stall for hundreds of
                        # microseconds per occurrence on real HW.
                        rln = wk4.tile([128, HPC, 128], f32, tag="rln")
                        nc.scalar.activation(
                            rln[64:65, :, :], av[64:65, :, :], Ln,
                            bias=0.0, scale=1.0,
                        )
                        nc.scalar.activation(
                            rzt[64:65, :, :], rln[64:65, :, :], Exp,
                            bias=0.0, scale=-1.0,
                        )
                    # broadcast 1/z to 64 rows (PE outer product, or gpsimd
                    # partition_broadcast), then normalize each head's [64, q]
                    # tile into bf16.  The PSUM bank also hosts the odd-head
                    # partition-shift staging on partitions 64..127.
                    bsh = bcp.tile([128, HPC, 128], f32, tag="bc")
                    bcs = wk2.tile([64, HPC, 128], bf16, tag="bcs")
                    if KNOB_TAILX in ("nobc", "nonorm"):
                        nc.vector.memset(bcs[:], 1.0)
                    elif KNOB_PB:
                        nc.gpsimd.partition_broadcast(
                            bcs[:], rzt[64:65, :, :], channels=64
                        )
                    else:
                        for h in range(HPC):
                            nc.tensor.matmul(
                                bsh[0:64, h, :], ones64[64:65, :],
                                rzt[64:65, h, :],
                                start=True, stop=True, skip_group_check=True,
                            )
                        nc.vector.tensor_copy(bcs[:], bsh[0:64, :, :])
                    # pair-stacked aoT for a K=128 wO: even heads' normalized
                    # tiles are written in place (partitions 0..63); odd heads
                    # go to a scratch tile and a PE identity-matmul moves both
                    # to partitions 64..127 (compute engines are lane-locked,
                    # but the PE can write any PSUM partition range)
                    if KNOB_SHIFT:
                        aoT = wk2.tile([128, 2, 128], bf16, tag="aoT")
                        if KNOB_SHIFT == 2:
                            aodd = aoddp[:, qi % 2, :, :]
                        else:
                            aodd_t = wk2.tile([64, 2, 128], bf16, tag="aodd")
                            aodd = aodd_t[:]
                        for pair in range(2):
                            nc.vector.tensor_mul(
                                aodd[0:64, pair, :], av[0:64, 2 * pair + 1, :],
                                bcs[:, 2 * pair + 1, :],
                            )
                        for pair in range(2):
                            if KNOB_SHIFT == 2:
                                nc.tensor.matmul(
                                    bsh[64:128, pair, :], ident16[:, 0:64],
                                    aodd[:, pair, :],
                                    start=True, stop=True,
                                    skip_group_check=True,
                                )
                            else:
                                nc.tensor.matmul(
                                    bsh[64:128, pair, :], ident16[0:64, 0:64],
                                    aodd[0:64, pair, :],
                                    start=True, stop=True,
                                    skip_group_check=True,
                                )
                        nc.vector.tensor_copy(
                            aoT[64:128, :, :], bsh[64:128, 0:2, :]
                        )
                        for pair in range(2):
                            nc.vector.tensor_mul(
                                aoT[0:64, pair, :], av[0:64, 2 * pair, :],
                                bcs[:, 2 * pair, :],
                            )
                    else:
                        aoT = wk2.tile([64, HPC, 128], bf16, tag="aoT")
                        for h in range(HPC):
                            nc.vector.tensor_mul(
                                aoT[:, h, :], av[0:64, h, :], bcs[:, h, :]
                            )
                    return aoT

                def emit_wo(qi, aoT):
                    for e in range(2):
                        ops = opp.tile([128, 512], f32, tag="o")
                        if KNOB_SHIFT:
                            for pair in range(2):
                                nc.tensor.matmul(
                                    ops[:], aoT[:, pair, :],
                                    wo[:, pair, ts(e, 512)],
                                    start=(pair == 0), stop=(pair == 1),
                                    skip_group_check=True,
                                )
                        else:
                            for h in range(HPC):
                                nc.tensor.matmul(
                                    ops[:], aoT[:, h, :], wo4[:, h, ts(e, 512)],
                                    start=(h == 0), stop=(h == 3),
                                    skip_group_check=True,
                                )
                        osb = wk2.tile([128, 512], bf16, tag="osb")
                        nc.vector.tensor_copy(osb[:], ops[:])
                        nc.sync.dma_start(out_d[ts(qi, 128), ts(e, 512)], osb[:])

                def emit_scores(qi, h, nkt, diag_kt, customs):
                    """QK^T + exp for one (q-tile, head); returns the P tile."""
                    pair, doff = h // 2, (h % 2) * 64
                    p_sb = wkp.tile([128, KT_TILES, 128], bf16, tag="p")
                    for kb in range(0, nkt, 8):
                        kw = min(8, nkt - kb)
                        st_ps = stp.tile([128, 8, 128], f32, tag="st")
                        for j in range(kw):
                            nc.tensor.matmul(
                                st_ps[:, j, :],
                                KT[doff : doff + 64, pair, ts(kb + j, 128)],
                                QT[doff : doff + 64, pair, ts(qi, 128)],
                                start=True, stop=True,
                                skip_group_check=True,
                            )
                        for kt, mi in customs:
                            if kb <= kt < kb + kw:
                                nc.vector.tensor_add(
                                    st_ps[:, kt - kb, :],
                                    st_ps[:, kt - kb, :],
                                    masks[:, mi, :],
                                )
                        # exp(score - 20); the shift cancels in the
                        # softmax normalization
                        nc.scalar.activation(
                            p_sb[:, kb : kb + kw, :], st_ps[:, 0:kw, :],
                            Exp, bias=cbias[:], scale=1.0,
                        )
                        if diag_kt is not None and kb <= diag_kt < kb + kw:
                            # zero P where k > q
                            if KNOB_AFF:
                                nc.gpsimd.affine_select(
                                    p_sb[:, diag_kt, :], p_sb[:, diag_kt, :],
                                    pattern=[[1, 128]],
                                    compare_op=mybir.AluOpType.is_ge,
                                    fill=0.0,
                                    base=0,
                                    channel_multiplier=-1,
                                )
                            else:
                                # gpsimd standard-library TensorTensor: the
                                # Pool queue is idle during attention
                                nc.gpsimd.tensor_mul(
                                    p_sb[:, diag_kt, :], p_sb[:, diag_kt, :],
                                    triu[:],
                                )
                    return p_sb

                def emit_av(av, h, nkt, p_sb):
                    for kt in range(nkt):
                        nc.tensor.matmul(
                            av[:, h, :],
                            vV[:, kt, h, :],
                            p_sb[:, kt, :],
                            start=(kt == 0), stop=(kt == nkt - 1),
                            skip_group_check=True,
                        )

                # slot-based software pipeline over the 64 (q-tile, head)
                # iterations: AV lags its scores by 2 slots (hides the Act exp
                # latency), each tile's normalize chain runs one slot after its
                # last AV, and its wO two slots later -- so no engine queue
                # ever sits on a cross-engine dependency.
                slot = 0
                due = []  # (due_slot, fn), FIFO per slot

                def sched(delay, fn):
                    due.append((slot + delay, fn))

                def run_due():
                    i = 0
                    while i < len(due):
                        s, fn = due[i]
                        if s <= slot:
                            due.pop(i)
                            fn()
                        else:
                            i += 1

                def make_av(av, h, nkt, p_sb):
                    return lambda: emit_av(av, h, nkt, p_sb)

                def make_norm(qi, av, box):
                    def fn():
                        box.append(emit_norm(qi, av))

                    return fn

                def make_wo(qi, box):
                    return lambda: emit_wo(qi, box[0])

                # alternate big/small tiles to smooth the engine mix
                _ord = []
                _big = sorted(range(QT_TILES), key=lambda q: -plan[q][0])
                for _i in range(QT_TILES // 2):
                    _ord += [_big[_i], _big[QT_TILES - 1 - _i]]
                for qi in _ord:
                    nkt, diag_kt, customs = plan[qi]
                    av = avp.tile([65, HPC, 128], f32, tag="av")
                    for h in range(HPC):
                        p_sb = emit_scores(qi, h, nkt, diag_kt, customs)
                        if KNOB_STOP != "qk":
                            sched(2, make_av(av, h, nkt, p_sb))
                        if h == 3 and not KNOB_STOP:
                            box = []
                            sched(3, make_norm(qi, av, box))
                            sched(5, make_wo(qi, box))
                        slot += 1
                        run_due()
                # flush
                slot += 16
                run_due()
                if KNOB_STOP in ("qk", "av"):
                    dumb2 = sbp.tile([128, 512], bf16)
                    nc.vector.tensor_copy(dumb2[:], QT[:, 0, 0:512])
                    nc.sync.dma_start(out_d[0:128, 0:512], dumb2[:])

    return nc


def _make_runner(plan, nt, repeat=1):
    """Compile the graph once and return fn(in_maps) -> list of out arrays."""
    import jax
    import numpy as np
    from jax.sharding import Mesh, PartitionSpec
    from jax.experimental.shard_map import shard_map
    import concourse.bass2jax as bass2jax
    import concourse.mybir as mybir

    nc = _build(plan, nt, repeat=repeat)
    bass2jax.install_neuronx_cc_hook()

    partition_name = nc.partition_id_tensor.name if nc.partition_id_tensor else None
    in_names, out_names, out_avals = [], [], []
    for alloc in nc.m.functions[0].allocations:
        if not isinstance(alloc, mybir.MemoryLocationSet):
            continue
        name = alloc.memorylocations[0].name
        if alloc.kind == "ExternalInput":
            if name != partition_name:
                in_names.append(name)
        elif alloc.kind == "ExternalOutput":
            out_names.append(name)
            out_avals.append(
                jax.core.ShapedArray(
                    tuple(alloc.tensor_shape), mybir.dt.np(alloc.dtype)
                )
            )
    all_names = in_names + out_names
    if partition_name is not None:
        all_names = all_names + [partition_name]

    def _body(*args):
        operands = list(args)
        if partition_name is not None:
            operands.append(bass2jax.partition_id_tensor())
        outs = bass2jax._bass_exec_p.bind(
            *operands,
            out_avals=tuple(out_avals),
            in_names=tuple(all_names),
            out_names=tuple(out_names),
            lowering_input_output_aliases=(),
            sim_require_finite=True,
            sim_require_nnan=True,
            nc=nc,
        )
        return tuple(outs)

    devices = jax.devices()[:N_CORES]
    mesh = Mesh(np.asarray(devices), ("core",))
    SHARED = {"cflat", "routersT", "dmask"}
    in_specs = tuple(
        PartitionSpec() if n in SHARED else PartitionSpec("core") for n in in_names
    ) + (PartitionSpec("core"),) * len(out_names)
    sharded = jax.jit(
        shard_map(
            _body,
            mesh=mesh,
            in_specs=in_specs,
            out_specs=(PartitionSpec("core"),) * len(out_names),
            check_rep=False,
        ),
        keep_unused=True,
    )
    zeros = [
        np.zeros((N_CORES * a.shape[0], *a.shape[1:]), a.dtype) for a in out_avals
    ]

    def make_args(in_maps, device=False):
        arrs = []
        for n in in_names:
            if n in SHARED:
                arrs.append(np.asarray(in_maps[0][n]))
            else:
                arrs.append(
                    np.concatenate([np.asarray(m[n]) for m in in_maps], axis=0)
                )
        arrs += list(zeros)
        if device:
            from jax.sharding import NamedSharding

            for i, n in enumerate(in_names):
                sh = NamedSharding(
                    mesh, PartitionSpec() if n in SHARED else PartitionSpec("core")
                )
                arrs[i] = jax.device_put(arrs[i], sh)
            sh = NamedSharding(mesh, PartitionSpec("core"))
            for i in range(len(in_names), len(arrs)):
                arrs[i] = jax.device_put(arrs[i], sh)
        return arrs

    def run(in_maps):
        outs = sharded(*make_args(in_maps))
        res = np.asarray(outs[out_names.index("out")])
        return res.reshape(N_CORES, S, D)

    run.sharded = sharded
    run.make_args = make_args
    run.out_index = out_names.index("out")
    return run


def _prepare(inputs):
    """Host-side prep: mask plan + per-core input maps."""
    x = np.asarray(inputs["x"], np.float32)
    mask = np.asarray(inputs["mask"], bool)[0, 0]
    compress = np.asarray(inputs["compress_neurons"], np.float32)
    rQ = np.asarray(inputs["router_Q"], np.float32)
    rK = np.asarray(inputs["router_K"], np.float32)
    rV = np.asarray(inputs["router_V"], np.float32)
    wQ = np.asarray(inputs["wQ"], np.float32)
    wK = np.asarray(inputs["wK"], np.float32)
    wV = np.asarray(inputs["wV"], np.float32)
    wO = np.asarray(inputs["wO"], np.float32)

    plan, mtiles = _mask_plan(mask)
    nt = len(mtiles)

    # host-side shared prep
    import ml_dtypes

    bf = ml_dtypes.bfloat16
    cflat = np.ascontiguousarray(
        compress.transpose(1, 2, 0).reshape(8, 128, RANK * NCMP)
    ).astype(bf)  # [D, R, NC] -> d-tiles
    routersT = np.ascontiguousarray(
        np.stack([rQ, rK, rV]).transpose(2, 0, 1).reshape(8, 128, 48)
    ).astype(bf)
    wqT = wQ.T * np.float32(1.0 / np.sqrt(DH))  # fold 1/sqrt(dh) into Q
    wkT, wvT = wK.T, wV.T
    wOT = np.ascontiguousarray(wO.T).astype(bf)  # [D, E]

    in_maps = []
    for c in range(N_CORES):
        b, q4 = divmod(c, 4)
        hs = slice(HPC * q4 * DH, HPC * q4 * DH + HPC * DH)
        m = {
            "xT": np.ascontiguousarray(x[b, q4 * SHARD : (q4 + 1) * SHARD, :].T)
            .reshape(8, 128, SHARD)
            .astype(bf),
            "cflat": cflat,
            "routersT": routersT,
            "wqkvT": np.ascontiguousarray(
                np.stack([wqT[:, hs], wkT[:, hs], wvT[:, hs]])
            ).astype(bf),
            "wOT": np.ascontiguousarray(wOT[hs, :]).reshape(2, 128, D),
        }
        if nt:
            m["dmask"] = mtiles
        in_maps.append(m)
    return plan, nt, in_maps


def kernel(**inputs):
    plan, nt, in_maps = _prepare(inputs)
    key = (plan, nt)
    if key not in _RUNNERS:
        _RUNNERS[key] = _make_runner(plan, nt)
    res = _RUNNERS[key](in_maps)  # [8, S, D] bf16 partials
    out = np.empty((B, S, D), np.float32)
    for b in range(B):
        out[b] = res[4 * b : 4 * b + 4].astype(np.float32).sum(axis=0)
    return out
